# revision 15
# baseline (speedup 1.0000x reference)
"""Trainium2 Bass kernel for an nn.DecoderBlock (pre-LN GPT block).

Reference computation (per batch element, fp32):
    h  = LN(x; g1,be1);  q,k,v = per-head projections of h
    y  = causal-softmax(q k^T / sqrt(hd)) v ;  x1 = x + y @ w_proj + b_proj
    h2 = LN(x1; g2,be2); out = x1 + gelu_tanh(h2 @ w_fc + b_fc) @ w_cp + b_cp

Shapes: B=8, T=1024, D=768, H=12, HD=64, F=3072.

Strategy: pure data parallelism — batch element b runs on core b (B == n_cores
== 8); the decoder block is independent per batch element so no collectives are
needed.  On-chip, all activations are kept *feature-major* ([D, T]: features on
partitions, tokens on the free axis) so chained matmuls need no transposes:
    out^T[n, t] = sum_d W[d, n] * A^T[d, t]   (lhsT = W as stored, rhs = A^T)
Attention scores are computed transposed (S^T[t, q]) so the softmax-weighted
probabilities land directly in the [t, q] layout the P@V matmul needs as its
moving operand; the two heads sharing a 128-partition group issue their K=64
score matmuls back-to-back so the PE runs them concurrently in disjoint
row-groups.  The softmax denominator comes from augmenting V with a
ones-column (row HD of the PV output is sum_t P[t,q]).  Softmax max-subtraction
is skipped: post-LN scores are O(5) so fp32 exp cannot overflow.

The PE instruction stream in attention is pure matmuls: softmax normalization
runs entirely on GpSimd (partition_broadcast of the denominator row) + DVE
(reciprocal_approx_fast, multiply), so the PE never stalls on it.  LayerNorm
statistics are per-token sums gathered with ones-column matmuls; the per-token
scalar math runs 128-lane in a token-major layout reached via PE transposes,
and the results are broadcast across partitions by GpSimd.

Host-side prep (numpy): transpose x per core, fold LN affine (g,be) and all
biases into the weight matrices, pre-pack weights into DMA-contiguous tiles,
cast to bf16. Matmuls run in bf16 with fp32 PSUM accumulation; LN stats,
residuals and softmax denominators stay fp32.
"""

import numpy as np
import ml_dtypes

import concourse.bass as bass
import concourse.mybir as mybir
import concourse.tile as tile
from concourse import bacc

BF16 = mybir.dt.bfloat16
F32 = mybir.dt.float32
AF = mybir.ActivationFunctionType
OP = mybir.AluOpType

# Full-problem dimensions (hardcoded; harness contract).
B, T, D, H = 8, 1024, 768, 12
HD = D // H
F = 4 * D
EPS = 1e-5
N_CORES = 8


# --------------------------------------------------------------------------
# Bass program builder (parameterized so a small variant can be simulated)
# --------------------------------------------------------------------------
def build_decoder_nc(T=T, D=D, H=H, F=F, TQ=512, with_bias=False, eps=EPS,
                     gelu_func=AF.Gelu_apprx_tanh):
    """Build the single-core Bass program (same program runs SPMD on all cores).

    DRAM I/O layouts (all prepared host-side):
      xT    [D, T]             f32   x^T (feature-major)
      wq,wk [MC,128,KC,128]    bf16  packed lhsT tiles (LN1 affine folded in)
      wv    [128,KC,D]         bf16  rhs layout for token-major V
      wp    [MC,128,KC,128]    bf16  w_proj packed
      wf    [FC,128,KC,128]    bf16  w_fc packed (LN2 affine folded in)
      wc    [FC,128,MC,128]    bf16  w_cp packed fc-major (plain reshape)
      *_b   [1, N]             bf16  folded bias rows (only if with_bias)
      outT  [D, T]             f32   output^T
    """
    assert D % 128 == 0 and F % 128 == 0 and T % TQ == 0 and TQ % 128 == 0
    TS = min(512, T)           # token chunk for projections/LN stats
    assert T % TS == 0
    KC = D // 128          # contraction chunks over D
    FC = F // 128          # chunks over MLP hidden
    MC = D // 128          # output-feature chunks over D
    NT = T // 128          # key/token chunks of 128
    NQ = T // TQ           # query chunks of TQ
    ND = TQ // 128         # diagonal mask variants
    HPC = 128 // HD        # heads per 128-partition group (2 for HD=64)
    VS = HD + 1            # V columns per head incl. ones-column
    scale = 1.0 / np.sqrt(HD)
    assert H % HPC == 0 and 2 * NT <= 128

    nc = bacc.Bacc()

    # ---- DRAM I/O ----
    xT = nc.dram_tensor("xT", [D, T], F32, kind="ExternalInput")
    wq_d = nc.dram_tensor("wq", [MC, 128, KC, 128], BF16, kind="ExternalInput")
    wk_d = nc.dram_tensor("wk", [MC, 128, KC, 128], BF16, kind="ExternalInput")
    wv_d = nc.dram_tensor("wv", [128, KC, D], BF16, kind="ExternalInput")
    wp_d = nc.dram_tensor("wp", [MC, 128, KC, 128], BF16, kind="ExternalInput")
    wf_d = nc.dram_tensor("wf", [FC, 128, KC, 128], BF16, kind="ExternalInput")
    wc_d = nc.dram_tensor("wc", [FC, 128, MC, 128], BF16, kind="ExternalInput")
    bias_d = {}
    if with_bias:
        for nm, width in (("bq", D), ("bk", D), ("bv", D), ("bp", D),
                          ("bf", F), ("bc", D)):
            bias_d[nm] = nc.dram_tensor(nm, [1, width], BF16,
                                        kind="ExternalInput")
    outT = nc.dram_tensor("outT", [D, T], F32, kind="ExternalOutput")
    outT_t = outT[:].rearrange("(o p) t -> p o t", p=128)

    # ---- constants (embedded in the NEFF) ----
    ones_bf = nc.inline_tensor(np.ones((1, T), ml_dtypes.bfloat16), "ones_bf")
    onescol = nc.inline_tensor(np.ones((128, 1), ml_dtypes.bfloat16),
                               "onescol")
    ident_np = np.eye(128, dtype=np.float32)
    ident_c = nc.inline_tensor(ident_np, "ident_c")
    # triangular mask for the diagonal 128x128 score blocks: 1 if i <= j
    m_np = (np.arange(128)[:, None] <= np.arange(128)[None, :]).astype(
        ml_dtypes.bfloat16)
    masks_d = nc.inline_tensor(m_np, "masks")

    with tile.TileContext(nc) as tc:
        with (
            tc.tile_pool(name="persist", bufs=1) as pp,
            tc.tile_pool(name="wts", bufs=3) as wpool,
            tc.tile_pool(name="work", bufs=3) as wkp,
            tc.tile_pool(name="small", bufs=1) as sp,
            tc.tile_pool(name="ps", bufs=8, space="PSUM") as ps,
        ):
            # ---- persistent SBUF tensors ----
            X = pp.tile([128, KC, T], F32, tag="X", name="X")
            ALN = pp.tile([128, KC, T], BF16, tag="ALN", name="ALN")
            QT = pp.tile([128, KC, T], BF16, tag="QT", name="QT")
            KT = pp.tile([128, KC, T], BF16, tag="KT", name="KT")
            Vt = pp.tile([128, NT, H * VS], BF16, tag="Vt", name="Vt")
            YT = pp.tile([128, KC, T], BF16, tag="YT", name="YT")
            X1 = pp.tile([128, KC, T], F32, tag="X1", name="X1")

            onesb_s = None
            if with_bias:
                onesb_s = pp.tile([1, T], BF16, tag="onesb", name="onesb_s")
                nc.sync.dma_start(out=onesb_s, in_=ones_bf[:])
            onescol_s = pp.tile([128, 1], BF16, tag="onescol",
                                name="onescol_s")
            nc.sync.dma_start(out=onescol_s, in_=onescol[:])
            ident_s = pp.tile([128, 128], F32, tag="ident", name="ident_s")
            nc.sync.dma_start(out=ident_s, in_=ident_c[:])
            eps_p = pp.tile([128, 1], F32, tag="eps", name="eps_p")
            nc.vector.memset(eps_p, eps)
            masks_s = pp.tile([128, 128], BF16, tag="masks", name="masks_s")
            nc.sync.dma_start(out=masks_s, in_=masks_d[:])
            biases = {}
            for nm, dten in bias_d.items():
                bt = pp.tile(list(dten.shape), BF16, tag=nm, name=f"{nm}_s")
                nc.sync.dma_start(out=bt, in_=dten[:])
                biases[nm] = bt

            Xbf = pp.tile([128, KC, T], BF16, tag="Xbf", name="Xbf")
            X1bf = pp.tile([128, KC, T], BF16, tag="X1bf", name="X1bf")

            # ---- load x^T ----
            xT_t = xT[:].rearrange("(o p) t -> p o t", p=128)
            for kc in range(KC):
                nc.sync.dma_start(out=X[:, kc, :], in_=xT_t[:, kc, :])
                nc.gpsimd.tensor_copy(out=Xbf[:, kc, :], in_=X[:, kc, :])

            # ---- LayerNorm: dst = (src - mu) * rstd, cast bf16 ----
            # Per-token sums via ones-column matmuls; scalar math runs
            # 128-lane in token-major layout (PE transpose there and back);
            # GpSimd broadcasts the per-token factors across partitions.
            def layernorm(src, srcbf, dst):
                NJ = TS // 128
                for tci in range(T // TS):
                    tsl = slice(tci * TS, (tci + 1) * TS)
                    pmu = ps.tile([128, TS], F32, tag="ps", name="pmu")
                    psq = ps.tile([128, TS], F32, tag="ps", name="psq")
                    for kc in range(KC):
                        sqc = wkp.tile([128, TS], BF16, tag="sqc", bufs=3,
                                       name="sqc")
                        nc.scalar.activation(out=sqc, in_=srcbf[:, kc, tsl],
                                             func=AF.Square)
                        nc.tensor.matmul(
                            pmu[0:1, :], onescol_s[:], srcbf[:, kc, tsl],
                            start=(kc == 0), stop=(kc == KC - 1))
                        nc.tensor.matmul(
                            psq[0:1, :], onescol_s[:], sqc,
                            start=(kc == 0), stop=(kc == KC - 1))
                    # token-major stats for this half via PE transposes
                    stok = sp.tile([128, NJ, 2], F32, tag="stok", bufs=2,
                                   name="stok")
                    for s, pstat in ((0, pmu), (1, psq)):
                        srow = sp.tile([1, TS], F32, tag="srow", bufs=2,
                                       name="srow")
                        nc.vector.tensor_copy(out=srow, in_=pstat[0:1, :])
                        ptk = ps.tile([128, TS], F32, tag="ps", name="ptk")
                        for jj in range(NJ):
                            nc.tensor.transpose(
                                ptk[:, jj:jj + 1],
                                srow[0:1, jj * 128:(jj + 1) * 128],
                                ident_s[0:1, 0:1])
                        nc.vector.tensor_copy(out=stok[:, :, s],
                                              in_=ptk[:, 0:NJ])
                    nc.vector.tensor_scalar_mul(stok, stok, 1.0 / D)
                    mu = stok[:, :, 0]
                    m2 = stok[:, :, 1]
                    var_t = sp.tile([128, NJ], F32, tag="var_t", bufs=2,
                                    name="var_t")
                    nc.vector.tensor_tensor(var_t, mu, mu, OP.mult)
                    nc.vector.tensor_tensor(var_t, m2, var_t, OP.subtract)
                    nc.scalar.activation(out=var_t, in_=var_t, func=AF.Sqrt,
                                         bias=eps_p[:])
                    st2 = sp.tile([128, NJ, 2], F32, tag="st2", bufs=2,
                                  name="st2")
                    nc.vector.reciprocal_approx_fast(out=st2[:, :, 0],
                                                     in_=var_t)
                    nc.vector.tensor_tensor(st2[:, :, 1], mu, st2[:, :, 0],
                                            OP.mult)
                    nc.vector.tensor_scalar_mul(st2[:, :, 1], st2[:, :, 1],
                                                -1.0)
                    # back to row layout and broadcast across partitions
                    prow = ps.tile([128, TS], F32, tag="ps", name="prow")
                    nc.tensor.transpose(
                        prow[0:2 * NJ, 0:128],
                        st2.rearrange("p a b -> p (a b)"), ident_s[:])
                    rows16 = sp.tile([2 * NJ, 128], BF16, tag="rows16",
                                     bufs=2, name="rows16")
                    nc.vector.tensor_copy(out=rows16,
                                          in_=prow[0:2 * NJ, 0:128])
                    rows0 = sp.tile([1, 2 * NJ, 128], BF16, tag="rows",
                                    bufs=2, name="rows0")
                    nc.sync.dma_start(
                        out=rows0.rearrange("p a b -> p (a b)"),
                        in_=rows16[:])
                    # apply: dst = srcbf*rstd + (-mu*rstd), bf16 throughout
                    for jj in range(NJ):
                        j = tci * NJ + jj
                        tslj = slice(j * 128, (j + 1) * 128)
                        prep_r = wkp.tile([128, 128], BF16, tag="prep_r",
                                          bufs=2, name="prep_r")
                        nc.gpsimd.partition_broadcast(
                            prep_r, rows0[0:1, 2 * jj, :])
                        prep_n = wkp.tile([128, 128], BF16, tag="prep_n",
                                          bufs=2, name="prep_n")
                        nc.gpsimd.partition_broadcast(
                            prep_n, rows0[0:1, 2 * jj + 1, :])
                        tmp = wkp.tile([128, KC, 128], BF16, tag="lntmp",
                                       bufs=3, name="lntmp")
                        nc.vector.tensor_tensor(
                            tmp, srcbf[:, :, tslj],
                            prep_r[:, None, :].to_broadcast((128, KC, 128)),
                            OP.mult)
                        nc.vector.tensor_tensor(
                            dst[:, :, tslj], tmp,
                            prep_n[:, None, :].to_broadcast((128, KC, 128)),
                            OP.add)

            layernorm(X, Xbf, ALN)

            # ---- QKV projections ----
            def bias_mm(psum, bias_t, msl, tsl):
                """Start `psum` with the rank-1 bias contribution; returns the
                start flag for the following contraction matmuls."""
                if bias_t is None:
                    return True
                nc.tensor.matmul(psum, bias_t[0:1, msl], onesb_s[0:1, tsl],
                                 start=True, stop=False)
                return False

            for mc in range(MC):
                msl = slice(mc * 128, (mc + 1) * 128)
                for nm, wten, dst in (("bq", wq_d, QT), ("bk", wk_d, KT)):
                    wt = wpool.tile([128, KC, 128], BF16, tag="w_qk", bufs=3,
                                    name="wt_qk")
                    nc.sync.dma_start(out=wt, in_=wten[mc])
                    for tci in range(T // TS):
                        tsl = slice(tci * TS, (tci + 1) * TS)
                        pq = ps.tile([128, TS], F32, tag="ps", name="pq")
                        st = bias_mm(pq, biases.get(nm), msl, tsl)
                        for kc in range(KC):
                            nc.tensor.matmul(
                                pq, wt[:, kc, :], ALN[:, kc, tsl],
                                start=st and (kc == 0), stop=(kc == KC - 1))
                        # cast to bf16; fold 1/sqrt(hd) into Q
                        nc.scalar.mul(dst[:, mc, tsl], pq[:],
                                      scale if dst is QT else 1.0)

            # V token-major: V[t, d] = sum_k A_ln[k, t] * wv[k, d] (+ bv)
            wv_t = pp.tile([128, KC, D], BF16, tag="wv", name="wv_t")
            nc.sync.dma_start(out=wv_t, in_=wv_d[:])
            for h in range(H):
                nc.vector.memset(Vt[:, :, h * VS + HD: h * VS + HD + 1], 1.0)
            NSL = D // 2 if D > 512 else D  # 384 for D=768: 6 heads per slice
            assert NSL % HD == 0
            for tch in range(NT):
                t128 = slice(tch * 128, (tch + 1) * 128)
                for si in range(D // NSL):
                    nsl = slice(si * NSL, (si + 1) * NSL)
                    pv = ps.tile([128, 512], F32, tag="ps", name="pv")
                    pvs = pv[:, :NSL]
                    st = True
                    if with_bias:
                        nc.tensor.matmul(pvs, onesb_s[0:1, 0:128],
                                         biases["bv"][0:1, nsl],
                                         start=True, stop=False)
                        st = False
                    for kc in range(KC):
                        nc.tensor.matmul(
                            pvs, ALN[:, kc, t128], wv_t[:, kc, nsl],
                            start=st and (kc == 0), stop=(kc == KC - 1))
                    h0 = si * (NSL // HD)
                    dst = Vt[:, tch, h0 * VS: (h0 + NSL // HD) * VS]
                    dst = dst.rearrange("p (h c) -> p h c", c=VS)[:, :, 0:HD]
                    nc.vector.tensor_copy(
                        out=dst, in_=pvs.rearrange("p (h c) -> p h c", c=HD))

            # ---- attention ----
            # S^T[t, q] = sum_hd K^T[hd, t] Q^T[hd, q]  (K=HD contraction).
            # The HPC heads of one partition group issue their score matmuls
            # back-to-back: disjoint PE row-groups run them concurrently.
            def softmax_norm(py, h, mc, half, qsl):
                """y^T[hd,q] = py[hd,q] / py[HD,q], written to YT — via
                GpSimd broadcast + DVE approx-reciprocal; no PE involvement."""
                hsl = slice(half * HD, (half + 1) * HD)
                den = wkp.tile([1, TQ], F32, tag="den", bufs=2, name="den")
                nc.scalar.copy(out=den, in_=py[HD: HD + 1, :])
                rep = wkp.tile([HD, TQ], F32, tag="rep", bufs=3, name="rep")
                nc.gpsimd.partition_broadcast(rep, den[:])
                rrec = wkp.tile([HD, TQ], F32, tag="rrec", bufs=3, name="rrec")
                nc.vector.reciprocal_approx_fast(out=rrec, in_=rep)
                nc.vector.tensor_tensor(
                    YT[hsl, mc, qsl], py[:HD, :], rrec[:], OP.mult)

            def attn_block(pys, mc, qc, tch):
                """scores+exp+mask+PV for one (pair, qc, key-chunk)."""
                qsl = slice(qc * TQ, (qc + 1) * TQ)
                tc_lo_diag = qc * TQ // 128
                tc_hi = (qc + 1) * TQ // 128
                t128 = slice(tch * 128, (tch + 1) * 128)
                dq = max(0, tch - tc_lo_diag) * 128
                rq = slice(dq, TQ)
                qslr = slice(qc * TQ + dq, (qc + 1) * TQ)
                pscs = []
                for half in range(HPC):
                    hsl = slice(half * HD, (half + 1) * HD)
                    psc = ps.tile([128, TQ], F32, tag="ps",
                                  name=f"psc{half}")
                    nc.tensor.matmul(
                        psc[:, rq], KT[hsl, mc, t128],
                        QT[hsl, mc, qslr], start=True, stop=True)
                    pscs.append(psc)
                for half in range(HPC):
                    h = mc * HPC + half
                    pexp = wkp.tile([128, TQ], BF16, tag="pexp",
                                    bufs=4, name="pexp")
                    nc.scalar.activation(out=pexp[:, rq],
                                         in_=pscs[half][:, rq],
                                         func=AF.Exp)
                    if tch >= tc_lo_diag:
                        nc.vector.tensor_tensor(
                            pexp[:, dq:dq + 128],
                            pexp[:, dq:dq + 128], masks_s[:], OP.mult)
                    nc.tensor.matmul(
                        pys[half][:VS, rq],
                        Vt[:, tch, h * VS: (h + 1) * VS], pexp[:, rq],
                        start=(tch == 0), stop=(tch == tc_hi - 1))

            # Interleave the light lower-triangular qc=0 work into qc=1's
            # stream so the PE never starves on the softmax chains.
            assert NQ in (1, 2)
            for mc in range(H // HPC):
                pys = {qc: [ps.tile([128, TQ], F32, tag="ps",
                                    name=f"py{qc}_{half}")
                            for half in range(HPC)]
                       for qc in range(NQ)}
                last_tc = T // 128 if NQ == 1 else 2 * TQ // 128
                for tch in range(last_tc if NQ == 2 else T // 128):
                    if NQ == 2:
                        if tch < TQ // 128:
                            attn_block(pys[0], mc, 0, tch)
                        attn_block(pys[1], mc, 1, tch)
                    else:
                        attn_block(pys[0], mc, 0, tch)
                for qc in range(NQ):
                    qsl = slice(qc * TQ, (qc + 1) * TQ)
                    for half in range(HPC):
                        softmax_norm(pys[qc][half], mc * HPC + half, mc,
                                     half, qsl)

            # ---- attn out-projection + residual (tci outer so X1 halves
            # complete early and LN2 statistics can overlap) ----
            for tci in range(T // TS):
                tsl = slice(tci * TS, (tci + 1) * TS)
                for mc in range(MC):
                    msl = slice(mc * 128, (mc + 1) * 128)
                    wt = wpool.tile([128, KC, 128], BF16, tag="w_p", bufs=3,
                                    name="wt_p")
                    nc.sync.dma_start(out=wt, in_=wp_d[mc])
                    po = ps.tile([128, TS], F32, tag="ps", name="po")
                    st = bias_mm(po, biases.get("bp"), msl, tsl)
                    for kc in range(KC):
                        nc.tensor.matmul(
                            po, wt[:, kc, :], YT[:, kc, tsl],
                            start=st and (kc == 0), stop=(kc == KC - 1))
                    nc.vector.tensor_tensor(
                        X1[:, mc, tsl], X[:, mc, tsl], po[:], OP.add)
                    nc.gpsimd.tensor_copy(out=X1bf[:, mc, tsl],
                                          in_=X1[:, mc, tsl])

            # ---- LN2 (A2 reuses ALN's slot; ALN is dead after QKV) ----
            A2 = pp.tile([128, KC, T], BF16, tag="ALN", name="A2")
            layernorm(X1, X1bf, A2)

            # ---- MLP: fc+gelu feeding cp accumulators, per 512-token half --
            # PSUM: MC pc accumulators held + 2 ph cycling = 8 banks exactly.
            for qc in range(T // TS):
                tsl = slice(qc * TS, (qc + 1) * TS)
                pcs = []
                for mc in range(MC):
                    pc = ps.tile([128, TS], F32, tag="ps", name=f"pc{mc}")
                    st = bias_mm(pc, biases.get("bc"),
                                 slice(mc * 128, (mc + 1) * 128), tsl)
                    pcs.append((pc, st))
                for fc in range(FC):
                    fsl = slice(fc * 128, (fc + 1) * 128)
                    wt = wpool.tile([128, KC, 128], BF16, tag="w_f", bufs=3,
                                    name="wt_f")
                    nc.sync.dma_start(out=wt, in_=wf_d[fc])
                    ph = ps.tile([128, TS], F32, tag="ps", name="ph")
                    st = bias_mm(ph, biases.get("bf"), fsl, tsl)
                    for kc in range(KC):
                        nc.tensor.matmul(
                            ph, wt[:, kc, :], A2[:, kc, tsl],
                            start=st and (kc == 0), stop=(kc == KC - 1))
                    hgel = wkp.tile([128, TS], BF16, tag="hgel", bufs=3,
                                    name="hgel")
                    nc.scalar.activation(out=hgel, in_=ph, func=gelu_func)
                    wtc = wpool.tile([128, MC, 128], BF16, tag="w_c", bufs=3,
                                     name="wt_c")
                    nc.sync.dma_start(out=wtc, in_=wc_d[fc])
                    for mc in range(MC):
                        pc, st = pcs[mc]
                        nc.tensor.matmul(
                            pc, wtc[:, mc, :], hgel,
                            start=st and (fc == 0), stop=(fc == FC - 1))
                for mc in range(MC):
                    pc, _ = pcs[mc]
                    ot = wkp.tile([128, TS], F32, tag="ot", bufs=3, name="ot")
                    nc.vector.tensor_tensor(ot, X1[:, mc, tsl], pc[:], OP.add)
                    nc.sync.dma_start(out=outT_t[:, mc, tsl], in_=ot)

    nc.finalize()
    return nc


# --------------------------------------------------------------------------
# Host-side input prep
# --------------------------------------------------------------------------
def _pack_lhsT(w):
    """[Dk, N] -> [N//128, 128, Dk//128, 128] contiguous lhsT tiles."""
    Dk, N = w.shape
    return np.ascontiguousarray(
        w.reshape(Dk // 128, 128, N // 128, 128).transpose(2, 1, 0, 3))


def prepare_weights(wq, bq, wk, bk, wv, bv, w_proj, b_proj, g1, be1, g2, be2,
                    w_fc, b_fc, w_cp, b_cp):
    """Fold LN affines + reshape heads; return packed bf16 arrays."""
    bf = ml_dtypes.bfloat16
    H_, D_, HD_ = wq.shape
    # [H, D, HD] -> [D, H*HD]
    wq2 = wq.transpose(1, 0, 2).reshape(D_, H_ * HD_).astype(np.float64)
    wk2 = wk.transpose(1, 0, 2).reshape(D_, H_ * HD_).astype(np.float64)
    wv2 = wv.transpose(1, 0, 2).reshape(D_, H_ * HD_).astype(np.float64)
    g1 = g1.astype(np.float64); be1 = be1.astype(np.float64)
    g2 = g2.astype(np.float64); be2 = be2.astype(np.float64)
    w_fc64 = w_fc.astype(np.float64)
    # fold LN affine: LN_aff(x) = n(x)*g + be  =>  W' = g[:,None]*W,
    # b' = b + be @ W
    arrs = {
        "wq": _pack_lhsT((g1[:, None] * wq2).astype(bf)),
        "wk": _pack_lhsT((g1[:, None] * wk2).astype(bf)),
        "wv": np.ascontiguousarray(
            (g1[:, None] * wv2).astype(bf)
            .reshape(-1, 128, wv2.shape[1]).transpose(1, 0, 2)),
        "wp": _pack_lhsT(w_proj.astype(bf)),
        "wf": _pack_lhsT((g2[:, None] * w_fc64).astype(bf)),
        "wc": np.ascontiguousarray(
            w_cp.astype(bf).reshape(-1, 128, w_cp.shape[1] // 128, 128)),
    }
    bias_arrs = {
        "bq": bq.reshape(-1).astype(np.float64) + be1 @ wq2,
        "bk": bk.reshape(-1).astype(np.float64) + be1 @ wk2,
        "bv": bv.reshape(-1).astype(np.float64) + be1 @ wv2,
        "bp": b_proj.astype(np.float64),
        "bf": b_fc.astype(np.float64) + be2 @ w_fc64,
        "bc": b_cp.astype(np.float64),
    }
    any_bias = bool(any(np.any(v != 0) for v in bias_arrs.values()))
    if any_bias:
        for k, v in bias_arrs.items():
            arrs[k] = v.astype(bf).reshape(1, -1)
    return arrs, any_bias


_NC_CACHE = {}


def kernel(**inputs):
    x = np.asarray(inputs["x"], np.float32)
    arrs, any_bias = prepare_weights(
        *(np.asarray(inputs[k]) for k in (
            "wq", "bq", "wk", "bk", "wv", "bv", "w_proj", "b_proj",
            "g1", "be1", "g2", "be2", "w_fc", "b_fc", "w_cp", "b_cp")))
    key = ("full", any_bias)
    if key not in _NC_CACHE:
        _NC_CACHE[key] = build_decoder_nc(with_bias=any_bias)
    nc = _NC_CACHE[key]

    in_maps = []
    for b in range(N_CORES):
        m = dict(arrs)
        m["xT"] = np.ascontiguousarray(x[b].T)
        in_maps.append(m)

    from concourse.bass_utils import run_bass_kernel_spmd
    res = run_bass_kernel_spmd(nc, in_maps, list(range(N_CORES)))
    out = np.stack([res.results[i]["outT"].T for i in range(N_CORES)])
    return out.astype(np.float32)


# revision 16
# speedup vs baseline: 1.0068x; 1.0068x over previous
"""Trainium2 Bass kernel for an nn.DecoderBlock (pre-LN GPT block).

Reference computation (per batch element, fp32):
    h  = LN(x; g1,be1);  q,k,v = per-head projections of h
    y  = causal-softmax(q k^T / sqrt(hd)) v ;  x1 = x + y @ w_proj + b_proj
    h2 = LN(x1; g2,be2); out = x1 + gelu_tanh(h2 @ w_fc + b_fc) @ w_cp + b_cp

Shapes: B=8, T=1024, D=768, H=12, HD=64, F=3072.

Strategy: pure data parallelism — batch element b runs on core b (B == n_cores
== 8); the decoder block is independent per batch element so no collectives are
needed.  On-chip, all activations are kept *feature-major* ([D, T]: features on
partitions, tokens on the free axis) so chained matmuls need no transposes:
    out^T[n, t] = sum_d W[d, n] * A^T[d, t]   (lhsT = W as stored, rhs = A^T)
Attention scores are computed transposed (S^T[t, q]) so the softmax-weighted
probabilities land directly in the [t, q] layout the P@V matmul needs as its
moving operand; the two heads sharing a 128-partition group issue their K=64
score matmuls back-to-back so the PE runs them concurrently in disjoint
row-groups.  The softmax denominator comes from augmenting V with a
ones-column (row HD of the PV output is sum_t P[t,q]).  Softmax max-subtraction
is skipped: post-LN scores are O(5) so fp32 exp cannot overflow.

The PE instruction stream in attention is pure matmuls: softmax normalization
runs entirely on GpSimd (partition_broadcast of the denominator row) + DVE
(reciprocal_approx_fast, multiply), so the PE never stalls on it.  LayerNorm
statistics are per-token sums gathered with ones-column matmuls; the per-token
scalar math runs 128-lane in a token-major layout reached via PE transposes,
and the results are broadcast across partitions by GpSimd.

Host-side prep (numpy): transpose x per core, fold LN affine (g,be) and all
biases into the weight matrices, pre-pack weights into DMA-contiguous tiles,
cast to bf16. Matmuls run in bf16 with fp32 PSUM accumulation; LN stats,
residuals and softmax denominators stay fp32.
"""

import numpy as np
import ml_dtypes

import concourse.bass as bass
import concourse.mybir as mybir
import concourse.tile as tile
from concourse import bacc

BF16 = mybir.dt.bfloat16
F32 = mybir.dt.float32
AF = mybir.ActivationFunctionType
OP = mybir.AluOpType

# Full-problem dimensions (hardcoded; harness contract).
B, T, D, H = 8, 1024, 768, 12
HD = D // H
F = 4 * D
EPS = 1e-5
N_CORES = 8


# --------------------------------------------------------------------------
# Bass program builder (parameterized so a small variant can be simulated)
# --------------------------------------------------------------------------
def build_decoder_nc(T=T, D=D, H=H, F=F, TQ=512, with_bias=False, eps=EPS,
                     gelu_func=AF.Gelu_apprx_tanh):
    """Build the single-core Bass program (same program runs SPMD on all cores).

    DRAM I/O layouts (all prepared host-side):
      xT    [D, T]             f32   x^T (feature-major)
      wq,wk [MC,128,KC,128]    bf16  packed lhsT tiles (LN1 affine folded in)
      wv    [128,KC,D]         bf16  rhs layout for token-major V
      wp    [MC,128,KC,128]    bf16  w_proj packed
      wf    [FC,128,KC,128]    bf16  w_fc packed (LN2 affine folded in)
      wc    [FC,128,MC,128]    bf16  w_cp packed fc-major (plain reshape)
      *_b   [1, N]             bf16  folded bias rows (only if with_bias)
      outT  [D, T]             f32   output^T
    """
    assert D % 128 == 0 and F % 128 == 0 and T % TQ == 0 and TQ % 128 == 0
    TS = min(512, T)           # token chunk for projections/LN stats
    assert T % TS == 0
    KC = D // 128          # contraction chunks over D
    FC = F // 128          # chunks over MLP hidden
    MC = D // 128          # output-feature chunks over D
    NT = T // 128          # key/token chunks of 128
    NQ = T // TQ           # query chunks of TQ
    ND = TQ // 128         # diagonal mask variants
    HPC = 128 // HD        # heads per 128-partition group (2 for HD=64)
    VS = HD + 1            # V columns per head incl. ones-column
    scale = 1.0 / np.sqrt(HD)
    assert H % HPC == 0 and 2 * NT <= 128

    nc = bacc.Bacc()

    # ---- DRAM I/O ----
    xT = nc.dram_tensor("xT", [D, T], F32, kind="ExternalInput")
    wq_d = nc.dram_tensor("wq", [MC, 128, KC, 128], BF16, kind="ExternalInput")
    wk_d = nc.dram_tensor("wk", [MC, 128, KC, 128], BF16, kind="ExternalInput")
    wv_d = nc.dram_tensor("wv", [128, KC, D], BF16, kind="ExternalInput")
    wp_d = nc.dram_tensor("wp", [MC, 128, KC, 128], BF16, kind="ExternalInput")
    wf_d = nc.dram_tensor("wf", [FC, 128, KC, 128], BF16, kind="ExternalInput")
    wc_d = nc.dram_tensor("wc", [FC, 128, MC, 128], BF16, kind="ExternalInput")
    bias_d = {}
    if with_bias:
        for nm, width in (("bq", D), ("bk", D), ("bv", D), ("bp", D),
                          ("bf", F), ("bc", D)):
            bias_d[nm] = nc.dram_tensor(nm, [1, width], BF16,
                                        kind="ExternalInput")
    outT = nc.dram_tensor("outT", [D, T], F32, kind="ExternalOutput")
    outT_t = outT[:].rearrange("(o p) t -> p o t", p=128)

    # ---- constants (embedded in the NEFF) ----
    ones_bf = nc.inline_tensor(np.ones((1, T), ml_dtypes.bfloat16), "ones_bf")
    onescol = nc.inline_tensor(np.ones((128, 1), ml_dtypes.bfloat16),
                               "onescol")
    ident_np = np.eye(128, dtype=np.float32)
    ident_c = nc.inline_tensor(ident_np, "ident_c")
    # triangular mask for the diagonal 128x128 score blocks: 1 if i <= j
    m_np = (np.arange(128)[:, None] <= np.arange(128)[None, :]).astype(
        ml_dtypes.bfloat16)
    masks_d = nc.inline_tensor(m_np, "masks")

    with tile.TileContext(nc) as tc:
        with (
            tc.tile_pool(name="persist", bufs=1) as pp,
            tc.tile_pool(name="wts", bufs=3) as wpool,
            tc.tile_pool(name="work", bufs=3) as wkp,
            tc.tile_pool(name="small", bufs=1) as sp,
            tc.tile_pool(name="ps", bufs=8, space="PSUM") as ps,
        ):
            # ---- persistent SBUF tensors ----
            X = pp.tile([128, KC, T], F32, tag="X", name="X")
            ALN = pp.tile([128, KC, T], BF16, tag="ALN", name="ALN")
            QT = pp.tile([128, KC, T], BF16, tag="QT", name="QT")
            KT = pp.tile([128, KC, T], BF16, tag="KT", name="KT")
            Vt = pp.tile([128, NT, H * VS], BF16, tag="Vt", name="Vt")
            YT = pp.tile([128, KC, T], BF16, tag="YT", name="YT")
            X1 = pp.tile([128, KC, T], F32, tag="X1", name="X1")

            onesb_s = None
            if with_bias:
                onesb_s = pp.tile([1, T], BF16, tag="onesb", name="onesb_s")
                nc.sync.dma_start(out=onesb_s, in_=ones_bf[:])
            onescol_s = pp.tile([128, 1], BF16, tag="onescol",
                                name="onescol_s")
            nc.sync.dma_start(out=onescol_s, in_=onescol[:])
            ident_s = pp.tile([128, 128], F32, tag="ident", name="ident_s")
            nc.sync.dma_start(out=ident_s, in_=ident_c[:])
            eps_p = pp.tile([128, 1], F32, tag="eps", name="eps_p")
            nc.vector.memset(eps_p, eps)
            masks_s = pp.tile([128, 128], BF16, tag="masks", name="masks_s")
            nc.sync.dma_start(out=masks_s, in_=masks_d[:])
            biases = {}
            for nm, dten in bias_d.items():
                bt = pp.tile(list(dten.shape), BF16, tag=nm, name=f"{nm}_s")
                nc.sync.dma_start(out=bt, in_=dten[:])
                biases[nm] = bt

            Xbf = pp.tile([128, KC, T], BF16, tag="Xbf", name="Xbf")
            X1bf = pp.tile([128, KC, T], BF16, tag="X1bf", name="X1bf")

            # ---- load x^T ----
            xT_t = xT[:].rearrange("(o p) t -> p o t", p=128)
            for kc in range(KC):
                nc.sync.dma_start(out=X[:, kc, :], in_=xT_t[:, kc, :])
                nc.gpsimd.tensor_copy(out=Xbf[:, kc, :], in_=X[:, kc, :])

            # ---- LayerNorm: dst = (src - mu) * rstd, cast bf16 ----
            # Per-token sums via ones-column matmuls; scalar math runs
            # 128-lane in token-major layout (PE transpose there and back);
            # GpSimd broadcasts the per-token factors across partitions.
            def layernorm(src, srcbf, dst):
                NJ = TS // 128
                for tci in range(T // TS):
                    tsl = slice(tci * TS, (tci + 1) * TS)
                    pmu = ps.tile([128, TS], F32, tag="ps", name="pmu")
                    psq = ps.tile([128, TS], F32, tag="ps", name="psq")
                    for kc in range(KC):
                        sqc = wkp.tile([128, TS], BF16, tag="sqc", bufs=3,
                                       name="sqc")
                        nc.scalar.activation(out=sqc, in_=srcbf[:, kc, tsl],
                                             func=AF.Square)
                        nc.tensor.matmul(
                            pmu[0:1, :], onescol_s[:], srcbf[:, kc, tsl],
                            start=(kc == 0), stop=(kc == KC - 1))
                        nc.tensor.matmul(
                            psq[0:1, :], onescol_s[:], sqc,
                            start=(kc == 0), stop=(kc == KC - 1))
                    # token-major stats for this half via PE transposes
                    stok = sp.tile([128, NJ, 2], F32, tag="stok", bufs=2,
                                   name="stok")
                    for s, pstat in ((0, pmu), (1, psq)):
                        srow = sp.tile([1, TS], F32, tag="srow", bufs=2,
                                       name="srow")
                        nc.vector.tensor_copy(out=srow, in_=pstat[0:1, :])
                        ptk = ps.tile([128, TS], F32, tag="ps", name="ptk")
                        for jj in range(NJ):
                            nc.tensor.transpose(
                                ptk[:, jj:jj + 1],
                                srow[0:1, jj * 128:(jj + 1) * 128],
                                ident_s[0:1, 0:1])
                        nc.vector.tensor_copy(out=stok[:, :, s],
                                              in_=ptk[:, 0:NJ])
                    nc.vector.tensor_scalar_mul(stok, stok, 1.0 / D)
                    mu = stok[:, :, 0]
                    m2 = stok[:, :, 1]
                    var_t = sp.tile([128, NJ], F32, tag="var_t", bufs=2,
                                    name="var_t")
                    nc.vector.tensor_tensor(var_t, mu, mu, OP.mult)
                    nc.vector.tensor_tensor(var_t, m2, var_t, OP.subtract)
                    nc.scalar.activation(out=var_t, in_=var_t, func=AF.Sqrt,
                                         bias=eps_p[:])
                    st2 = sp.tile([128, NJ, 2], F32, tag="st2", bufs=2,
                                  name="st2")
                    nc.vector.reciprocal_approx_fast(out=st2[:, :, 0],
                                                     in_=var_t)
                    nc.vector.tensor_tensor(st2[:, :, 1], mu, st2[:, :, 0],
                                            OP.mult)
                    nc.vector.tensor_scalar_mul(st2[:, :, 1], st2[:, :, 1],
                                                -1.0)
                    # back to row layout and broadcast across partitions
                    prow = ps.tile([128, TS], F32, tag="ps", name="prow")
                    nc.tensor.transpose(
                        prow[0:2 * NJ, 0:128],
                        st2.rearrange("p a b -> p (a b)"), ident_s[:])
                    rows16 = sp.tile([2 * NJ, 128], BF16, tag="rows16",
                                     bufs=2, name="rows16")
                    nc.vector.tensor_copy(out=rows16,
                                          in_=prow[0:2 * NJ, 0:128])
                    rows0 = sp.tile([1, 2 * NJ, 128], BF16, tag="rows",
                                    bufs=2, name="rows0")
                    nc.sync.dma_start(
                        out=rows0.rearrange("p a b -> p (a b)"),
                        in_=rows16[:])
                    # apply: dst = srcbf*rstd + (-mu*rstd), bf16 throughout
                    for jj in range(NJ):
                        j = tci * NJ + jj
                        tslj = slice(j * 128, (j + 1) * 128)
                        prep_r = wkp.tile([128, 128], BF16, tag="prep_r",
                                          bufs=2, name="prep_r")
                        nc.gpsimd.partition_broadcast(
                            prep_r, rows0[0:1, 2 * jj, :])
                        prep_n = wkp.tile([128, 128], BF16, tag="prep_n",
                                          bufs=2, name="prep_n")
                        nc.gpsimd.partition_broadcast(
                            prep_n, rows0[0:1, 2 * jj + 1, :])
                        tmp = wkp.tile([128, KC, 128], BF16, tag="lntmp",
                                       bufs=3, name="lntmp")
                        nc.vector.tensor_tensor(
                            tmp, srcbf[:, :, tslj],
                            prep_r[:, None, :].to_broadcast((128, KC, 128)),
                            OP.mult)
                        nc.vector.tensor_tensor(
                            dst[:, :, tslj], tmp,
                            prep_n[:, None, :].to_broadcast((128, KC, 128)),
                            OP.add)

            layernorm(X, Xbf, ALN)

            # ---- QKV projections ----
            def bias_mm(psum, bias_t, msl, tsl):
                """Start `psum` with the rank-1 bias contribution; returns the
                start flag for the following contraction matmuls."""
                if bias_t is None:
                    return True
                nc.tensor.matmul(psum, bias_t[0:1, msl], onesb_s[0:1, tsl],
                                 start=True, stop=False)
                return False

            for mc in range(MC):
                msl = slice(mc * 128, (mc + 1) * 128)
                for nm, wten, dst in (("bq", wq_d, QT), ("bk", wk_d, KT)):
                    wt = wpool.tile([128, KC, 128], BF16, tag="w_qk", bufs=3,
                                    name="wt_qk")
                    nc.sync.dma_start(out=wt, in_=wten[mc])
                    for tci in range(T // TS):
                        tsl = slice(tci * TS, (tci + 1) * TS)
                        pq = ps.tile([128, TS], F32, tag="ps", name="pq")
                        st = bias_mm(pq, biases.get(nm), msl, tsl)
                        for kc in range(KC):
                            nc.tensor.matmul(
                                pq, wt[:, kc, :], ALN[:, kc, tsl],
                                start=st and (kc == 0), stop=(kc == KC - 1))
                        # cast to bf16; fold 1/sqrt(hd) into Q
                        nc.scalar.mul(dst[:, mc, tsl], pq[:],
                                      scale if dst is QT else 1.0)

            # V token-major: V[t, d] = sum_k A_ln[k, t] * wv[k, d] (+ bv)
            wv_t = pp.tile([128, KC, D], BF16, tag="wv", name="wv_t")
            nc.sync.dma_start(out=wv_t, in_=wv_d[:])
            for h in range(H):
                nc.vector.memset(Vt[:, :, h * VS + HD: h * VS + HD + 1], 1.0)
            NSL = D // 2 if D > 512 else D  # 384 for D=768: 6 heads per slice
            assert NSL % HD == 0
            for tch in range(NT):
                t128 = slice(tch * 128, (tch + 1) * 128)
                for si in range(D // NSL):
                    nsl = slice(si * NSL, (si + 1) * NSL)
                    pv = ps.tile([128, 512], F32, tag="ps", name="pv")
                    pvs = pv[:, :NSL]
                    st = True
                    if with_bias:
                        nc.tensor.matmul(pvs, onesb_s[0:1, 0:128],
                                         biases["bv"][0:1, nsl],
                                         start=True, stop=False)
                        st = False
                    for kc in range(KC):
                        nc.tensor.matmul(
                            pvs, ALN[:, kc, t128], wv_t[:, kc, nsl],
                            start=st and (kc == 0), stop=(kc == KC - 1))
                    h0 = si * (NSL // HD)
                    dst = Vt[:, tch, h0 * VS: (h0 + NSL // HD) * VS]
                    dst = dst.rearrange("p (h c) -> p h c", c=VS)[:, :, 0:HD]
                    nc.vector.tensor_copy(
                        out=dst, in_=pvs.rearrange("p (h c) -> p h c", c=HD))

            # ---- attention ----
            # S^T[t, q] = sum_hd K^T[hd, t] Q^T[hd, q]  (K=HD contraction).
            # The HPC heads of one partition group issue their score matmuls
            # back-to-back: disjoint PE row-groups run them concurrently.
            def softmax_norm(py, h, mc, half, qsl):
                """y^T[hd,q] = py[hd,q] / py[HD,q], written to YT — via
                GpSimd broadcast + DVE approx-reciprocal; no PE involvement."""
                hsl = slice(half * HD, (half + 1) * HD)
                den = wkp.tile([1, TQ], F32, tag="den", bufs=2, name="den")
                nc.scalar.copy(out=den, in_=py[HD: HD + 1, :])
                rep = wkp.tile([HD, TQ], F32, tag="rep", bufs=3, name="rep")
                nc.gpsimd.partition_broadcast(rep, den[:])
                rrec = wkp.tile([HD, TQ], F32, tag="rrec", bufs=3, name="rrec")
                nc.vector.reciprocal_approx_fast(out=rrec, in_=rep)
                nc.vector.tensor_tensor(
                    YT[hsl, mc, qsl], py[:HD, :], rrec[:], OP.mult)

            def attn_block(pys, mc, qc, tch):
                """scores+exp+mask+PV for one (pair, qc, key-chunk)."""
                qsl = slice(qc * TQ, (qc + 1) * TQ)
                tc_lo_diag = qc * TQ // 128
                tc_hi = (qc + 1) * TQ // 128
                t128 = slice(tch * 128, (tch + 1) * 128)
                dq = max(0, tch - tc_lo_diag) * 128
                rq = slice(dq, TQ)
                qslr = slice(qc * TQ + dq, (qc + 1) * TQ)
                pscs = []
                for half in range(HPC):
                    hsl = slice(half * HD, (half + 1) * HD)
                    psc = ps.tile([128, TQ], F32, tag="ps",
                                  name=f"psc{half}")
                    nc.tensor.matmul(
                        psc[:, rq], KT[hsl, mc, t128],
                        QT[hsl, mc, qslr], start=True, stop=True)
                    pscs.append(psc)
                for half in range(HPC):
                    h = mc * HPC + half
                    pexp = wkp.tile([128, TQ], BF16, tag="pexp",
                                    bufs=4, name="pexp")
                    nc.scalar.activation(out=pexp[:, rq],
                                         in_=pscs[half][:, rq],
                                         func=AF.Exp)
                    if tch >= tc_lo_diag:
                        nc.vector.tensor_tensor(
                            pexp[:, dq:dq + 128],
                            pexp[:, dq:dq + 128], masks_s[:], OP.mult)
                    nc.tensor.matmul(
                        pys[half][:VS, rq],
                        Vt[:, tch, h * VS: (h + 1) * VS], pexp[:, rq],
                        start=(tch == 0), stop=(tch == tc_hi - 1))

            # Interleave the light lower-triangular qc=0 work into qc=1's
            # stream so the PE never starves on the softmax chains.
            assert NQ in (1, 2)
            NTQ = TQ // 128
            for mc in range(H // HPC):
                pys = {qc: [ps.tile([128, TQ], F32, tag="ps",
                                    name=f"py{qc}_{half}")
                            for half in range(HPC)]
                       for qc in range(NQ)}
                for tch in range(NQ * NTQ):
                    if NQ == 2 and tch < NTQ:
                        attn_block(pys[0], mc, 0, tch)
                    attn_block(pys[NQ - 1], mc, NQ - 1, tch)
                    if NQ == 2 and tch == NTQ - 1:
                        # qc=0 accumulators are complete: normalize now so
                        # their PSUM banks free before the next pair starts
                        for half in range(HPC):
                            softmax_norm(pys[0][half], mc * HPC + half, mc,
                                         half, slice(0, TQ))
                qc = NQ - 1
                qsl = slice(qc * TQ, (qc + 1) * TQ)
                for half in range(HPC):
                    softmax_norm(pys[qc][half], mc * HPC + half, mc,
                                 half, qsl)

            # ---- attn out-projection + residual (tci outer so X1 halves
            # complete early and LN2 statistics can overlap) ----
            for tci in range(T // TS):
                tsl = slice(tci * TS, (tci + 1) * TS)
                for mc in range(MC):
                    msl = slice(mc * 128, (mc + 1) * 128)
                    wt = wpool.tile([128, KC, 128], BF16, tag="w_p", bufs=3,
                                    name="wt_p")
                    nc.sync.dma_start(out=wt, in_=wp_d[mc])
                    po = ps.tile([128, TS], F32, tag="ps", name="po")
                    st = bias_mm(po, biases.get("bp"), msl, tsl)
                    for kc in range(KC):
                        nc.tensor.matmul(
                            po, wt[:, kc, :], YT[:, kc, tsl],
                            start=st and (kc == 0), stop=(kc == KC - 1))
                    nc.vector.tensor_tensor(
                        X1[:, mc, tsl], X[:, mc, tsl], po[:], OP.add)
                    nc.gpsimd.tensor_copy(out=X1bf[:, mc, tsl],
                                          in_=X1[:, mc, tsl])

            # ---- LN2 (A2 reuses ALN's slot; ALN is dead after QKV) ----
            A2 = pp.tile([128, KC, T], BF16, tag="ALN", name="A2")
            layernorm(X1, X1bf, A2)

            # ---- MLP: fc+gelu feeding cp accumulators, per 512-token half --
            # PSUM: MC pc accumulators held + 2 ph cycling = 8 banks exactly.
            for qc in range(T // TS):
                tsl = slice(qc * TS, (qc + 1) * TS)
                pcs = []
                for mc in range(MC):
                    pc = ps.tile([128, TS], F32, tag="ps", name=f"pc{mc}")
                    st = bias_mm(pc, biases.get("bc"),
                                 slice(mc * 128, (mc + 1) * 128), tsl)
                    pcs.append((pc, st))
                for fc in range(FC):
                    fsl = slice(fc * 128, (fc + 1) * 128)
                    wt = wpool.tile([128, KC, 128], BF16, tag="w_f", bufs=3,
                                    name="wt_f")
                    nc.sync.dma_start(out=wt, in_=wf_d[fc])
                    ph = ps.tile([128, TS], F32, tag="ps", name="ph")
                    st = bias_mm(ph, biases.get("bf"), fsl, tsl)
                    for kc in range(KC):
                        nc.tensor.matmul(
                            ph, wt[:, kc, :], A2[:, kc, tsl],
                            start=st and (kc == 0), stop=(kc == KC - 1))
                    hgel = wkp.tile([128, TS], BF16, tag="hgel", bufs=3,
                                    name="hgel")
                    nc.scalar.activation(out=hgel, in_=ph, func=gelu_func)
                    wtc = wpool.tile([128, MC, 128], BF16, tag="w_c", bufs=3,
                                     name="wt_c")
                    nc.sync.dma_start(out=wtc, in_=wc_d[fc])
                    for mc in range(MC):
                        pc, st = pcs[mc]
                        nc.tensor.matmul(
                            pc, wtc[:, mc, :], hgel,
                            start=st and (fc == 0), stop=(fc == FC - 1))
                for mc in range(MC):
                    pc, _ = pcs[mc]
                    ot = wkp.tile([128, TS], F32, tag="ot", bufs=3, name="ot")
                    nc.vector.tensor_tensor(ot, X1[:, mc, tsl], pc[:], OP.add)
                    nc.sync.dma_start(out=outT_t[:, mc, tsl], in_=ot)

    nc.finalize()
    return nc


# --------------------------------------------------------------------------
# Host-side input prep
# --------------------------------------------------------------------------
def _pack_lhsT(w):
    """[Dk, N] -> [N//128, 128, Dk//128, 128] contiguous lhsT tiles."""
    Dk, N = w.shape
    return np.ascontiguousarray(
        w.reshape(Dk // 128, 128, N // 128, 128).transpose(2, 1, 0, 3))


def prepare_weights(wq, bq, wk, bk, wv, bv, w_proj, b_proj, g1, be1, g2, be2,
                    w_fc, b_fc, w_cp, b_cp):
    """Fold LN affines + reshape heads; return packed bf16 arrays."""
    bf = ml_dtypes.bfloat16
    H_, D_, HD_ = wq.shape
    # [H, D, HD] -> [D, H*HD]
    wq2 = wq.transpose(1, 0, 2).reshape(D_, H_ * HD_).astype(np.float64)
    wk2 = wk.transpose(1, 0, 2).reshape(D_, H_ * HD_).astype(np.float64)
    wv2 = wv.transpose(1, 0, 2).reshape(D_, H_ * HD_).astype(np.float64)
    g1 = g1.astype(np.float64); be1 = be1.astype(np.float64)
    g2 = g2.astype(np.float64); be2 = be2.astype(np.float64)
    w_fc64 = w_fc.astype(np.float64)
    # fold LN affine: LN_aff(x) = n(x)*g + be  =>  W' = g[:,None]*W,
    # b' = b + be @ W
    arrs = {
        "wq": _pack_lhsT((g1[:, None] * wq2).astype(bf)),
        "wk": _pack_lhsT((g1[:, None] * wk2).astype(bf)),
        "wv": np.ascontiguousarray(
            (g1[:, None] * wv2).astype(bf)
            .reshape(-1, 128, wv2.shape[1]).transpose(1, 0, 2)),
        "wp": _pack_lhsT(w_proj.astype(bf)),
        "wf": _pack_lhsT((g2[:, None] * w_fc64).astype(bf)),
        "wc": np.ascontiguousarray(
            w_cp.astype(bf).reshape(-1, 128, w_cp.shape[1] // 128, 128)),
    }
    bias_arrs = {
        "bq": bq.reshape(-1).astype(np.float64) + be1 @ wq2,
        "bk": bk.reshape(-1).astype(np.float64) + be1 @ wk2,
        "bv": bv.reshape(-1).astype(np.float64) + be1 @ wv2,
        "bp": b_proj.astype(np.float64),
        "bf": b_fc.astype(np.float64) + be2 @ w_fc64,
        "bc": b_cp.astype(np.float64),
    }
    any_bias = bool(any(np.any(v != 0) for v in bias_arrs.values()))
    if any_bias:
        for k, v in bias_arrs.items():
            arrs[k] = v.astype(bf).reshape(1, -1)
    return arrs, any_bias


_NC_CACHE = {}


def kernel(**inputs):
    x = np.asarray(inputs["x"], np.float32)
    arrs, any_bias = prepare_weights(
        *(np.asarray(inputs[k]) for k in (
            "wq", "bq", "wk", "bk", "wv", "bv", "w_proj", "b_proj",
            "g1", "be1", "g2", "be2", "w_fc", "b_fc", "w_cp", "b_cp")))
    key = ("full", any_bias)
    if key not in _NC_CACHE:
        _NC_CACHE[key] = build_decoder_nc(with_bias=any_bias)
    nc = _NC_CACHE[key]

    in_maps = []
    for b in range(N_CORES):
        m = dict(arrs)
        m["xT"] = np.ascontiguousarray(x[b].T)
        in_maps.append(m)

    from concourse.bass_utils import run_bass_kernel_spmd
    res = run_bass_kernel_spmd(nc, in_maps, list(range(N_CORES)))
    out = np.stack([res.results[i]["outT"].T for i in range(N_CORES)])
    return out.astype(np.float32)


# revision 17
# speedup vs baseline: 1.0590x; 1.0518x over previous
"""Trainium2 Bass kernel for an nn.DecoderBlock (pre-LN GPT block).

Reference computation (per batch element, fp32):
    h  = LN(x; g1,be1);  q,k,v = per-head projections of h
    y  = causal-softmax(q k^T / sqrt(hd)) v ;  x1 = x + y @ w_proj + b_proj
    h2 = LN(x1; g2,be2); out = x1 + gelu_tanh(h2 @ w_fc + b_fc) @ w_cp + b_cp

Shapes: B=8, T=1024, D=768, H=12, HD=64, F=3072.

Strategy: pure data parallelism — batch element b runs on core b (B == n_cores
== 8); the decoder block is independent per batch element so no collectives are
needed.  On-chip, all activations are kept *feature-major* ([D, T]: features on
partitions, tokens on the free axis) so chained matmuls need no transposes:
    out^T[n, t] = sum_d W[d, n] * A^T[d, t]   (lhsT = W as stored, rhs = A^T)
Attention scores are computed transposed (S^T[t, q]) so the softmax-weighted
probabilities land directly in the [t, q] layout the P@V matmul needs as its
moving operand; the two heads sharing a 128-partition group issue their K=64
score matmuls back-to-back so the PE runs them concurrently in disjoint
row-groups.  The softmax denominator comes from augmenting V with a
ones-column (row HD of the PV output is sum_t P[t,q]).  Softmax max-subtraction
is skipped: post-LN scores are O(5) so fp32 exp cannot overflow.

The PE instruction stream in attention is pure matmuls: softmax normalization
runs entirely on GpSimd (partition_broadcast of the denominator row) + DVE
(reciprocal_approx_fast, multiply), so the PE never stalls on it.  LayerNorm
statistics are per-token sums gathered with ones-column matmuls; the per-token
scalar math runs 128-lane in a token-major layout reached via PE transposes,
and the results are broadcast across partitions by GpSimd.

Host-side prep (numpy): transpose x per core, fold LN affine (g,be) and all
biases into the weight matrices, pre-pack weights into DMA-contiguous tiles,
cast to bf16. Matmuls run in bf16 with fp32 PSUM accumulation; LN stats,
residuals and softmax denominators stay fp32.
"""

import numpy as np
import ml_dtypes

import concourse.bass as bass
import concourse.mybir as mybir
import concourse.tile as tile
from concourse import bacc

BF16 = mybir.dt.bfloat16
F32 = mybir.dt.float32
AF = mybir.ActivationFunctionType
OP = mybir.AluOpType

# Full-problem dimensions (hardcoded; harness contract).
B, T, D, H = 8, 1024, 768, 12
HD = D // H
F = 4 * D
EPS = 1e-5
N_CORES = 8


# --------------------------------------------------------------------------
# Bass program builder (parameterized so a small variant can be simulated)
# --------------------------------------------------------------------------
def build_decoder_nc(T=T, D=D, H=H, F=F, TQ=512, with_bias=False, eps=EPS,
                     gelu_func=AF.Gelu_apprx_tanh):
    """Build the single-core Bass program (same program runs SPMD on all cores).

    DRAM I/O layouts (all prepared host-side):
      xT    [D, T]             f32   x^T (feature-major)
      wq,wk [MC,128,KC,128]    bf16  packed lhsT tiles (LN1 affine folded in)
      wv    [128,KC,D]         bf16  rhs layout for token-major V
      wp    [MC,128,KC,128]    bf16  w_proj packed
      wf    [FC,128,KC,128]    bf16  w_fc packed (LN2 affine folded in)
      wc    [FC,128,MC,128]    bf16  w_cp packed fc-major (plain reshape)
      *_b   [1, N]             bf16  folded bias rows (only if with_bias)
      outT  [D, T]             f32   output^T
    """
    assert D % 128 == 0 and F % 128 == 0 and T % TQ == 0 and TQ % 128 == 0
    TS = min(512, T)           # token chunk for projections/LN stats
    assert T % TS == 0
    KC = D // 128          # contraction chunks over D
    FC = F // 128          # chunks over MLP hidden
    MC = D // 128          # output-feature chunks over D
    NT = T // 128          # key/token chunks of 128
    NQ = T // TQ           # query chunks of TQ
    ND = TQ // 128         # diagonal mask variants
    HPC = 128 // HD        # heads per 128-partition group (2 for HD=64)
    VS = HD + 1            # V columns per head incl. ones-column
    scale = 1.0 / np.sqrt(HD)
    assert H % HPC == 0 and 2 * NT <= 128

    nc = bacc.Bacc()

    # ---- DRAM I/O ----
    xT = nc.dram_tensor("xT", [D, T], F32, kind="ExternalInput")
    wq_d = nc.dram_tensor("wq", [MC, 128, KC, 128], BF16, kind="ExternalInput")
    wk_d = nc.dram_tensor("wk", [MC, 128, KC, 128], BF16, kind="ExternalInput")
    wv_d = nc.dram_tensor("wv", [128, KC, D], BF16, kind="ExternalInput")
    wp_d = nc.dram_tensor("wp", [MC, 128, KC, 128], BF16, kind="ExternalInput")
    wf_d = nc.dram_tensor("wf", [FC, 128, KC, 128], BF16, kind="ExternalInput")
    wc_d = nc.dram_tensor("wc", [FC, 128, MC, 128], BF16, kind="ExternalInput")
    bias_d = {}
    if with_bias:
        for nm, width in (("bq", D), ("bk", D), ("bv", D), ("bp", D),
                          ("bf", F), ("bc", D)):
            bias_d[nm] = nc.dram_tensor(nm, [1, width], BF16,
                                        kind="ExternalInput")
    outT = nc.dram_tensor("outT", [D, T], F32, kind="ExternalOutput")
    outT_t = outT[:].rearrange("(o p) t -> p o t", p=128)

    # ---- constants (embedded in the NEFF) ----
    ones_bf = nc.inline_tensor(np.ones((1, T), ml_dtypes.bfloat16), "ones_bf")
    onescol = nc.inline_tensor(np.ones((128, 1), ml_dtypes.bfloat16),
                               "onescol")
    ident_np = np.eye(128, dtype=np.float32)
    ident_c = nc.inline_tensor(ident_np, "ident_c")
    # triangular mask for the diagonal 128x128 score blocks: 1 if i <= j
    m_np = (np.arange(128)[:, None] <= np.arange(128)[None, :]).astype(
        ml_dtypes.bfloat16)
    masks_d = nc.inline_tensor(m_np, "masks")

    with tile.TileContext(nc) as tc:
        with (
            tc.tile_pool(name="persist", bufs=1) as pp,
            tc.tile_pool(name="wts", bufs=3) as wpool,
            tc.tile_pool(name="work", bufs=3) as wkp,
            tc.tile_pool(name="small", bufs=1) as sp,
            tc.tile_pool(name="ps", bufs=8, space="PSUM") as ps,
        ):
            # ---- persistent SBUF tensors ----
            X = pp.tile([128, KC, T], F32, tag="X", name="X")
            ALN = pp.tile([128, KC, T], BF16, tag="ALN", name="ALN")
            QT = pp.tile([128, KC, T], BF16, tag="QT", name="QT")
            KT = pp.tile([128, KC, T], BF16, tag="KT", name="KT")
            Vt = pp.tile([128, NT, H * VS], BF16, tag="Vt", name="Vt")
            YT = pp.tile([128, KC, T], BF16, tag="YT", name="YT")
            X1 = pp.tile([128, KC, T], F32, tag="X1", name="X1")

            onesb_s = None
            if with_bias:
                onesb_s = pp.tile([1, T], BF16, tag="onesb", name="onesb_s")
                nc.sync.dma_start(out=onesb_s, in_=ones_bf[:])
            onescol_s = pp.tile([128, 1], BF16, tag="onescol",
                                name="onescol_s")
            nc.sync.dma_start(out=onescol_s, in_=onescol[:])
            ident_s = pp.tile([128, 128], F32, tag="ident", name="ident_s")
            nc.sync.dma_start(out=ident_s, in_=ident_c[:])
            eps_p = pp.tile([128, 1], F32, tag="eps", name="eps_p")
            nc.vector.memset(eps_p, eps)
            masks_s = pp.tile([128, 128], BF16, tag="masks", name="masks_s")
            nc.sync.dma_start(out=masks_s, in_=masks_d[:])
            biases = {}
            for nm, dten in bias_d.items():
                bt = pp.tile(list(dten.shape), BF16, tag=nm, name=f"{nm}_s")
                nc.sync.dma_start(out=bt, in_=dten[:])
                biases[nm] = bt

            Xbf = pp.tile([128, KC, T], BF16, tag="Xbf", name="Xbf")
            X1bf = pp.tile([128, KC, T], BF16, tag="X1bf", name="X1bf")

            # ---- load x^T ----
            xT_t = xT[:].rearrange("(o p) t -> p o t", p=128)
            for kc in range(KC):
                nc.sync.dma_start(out=X[:, kc, :], in_=xT_t[:, kc, :])
                nc.gpsimd.tensor_copy(out=Xbf[:, kc, :], in_=X[:, kc, :])

            # ---- LayerNorm: dst = (src - mu) * rstd, cast bf16 ----
            # Per-token sums via ones-column matmuls; scalar math runs
            # 128-lane in token-major layout (PE transpose there and back);
            # GpSimd broadcasts the per-token factors across partitions.
            def layernorm(src, srcbf, dst):
                NJ = TS // 128
                for tci in range(T // TS):
                    tsl = slice(tci * TS, (tci + 1) * TS)
                    pmu = ps.tile([128, TS], F32, tag="ps", name="pmu")
                    psq = ps.tile([128, TS], F32, tag="ps", name="psq")
                    for kc in range(KC):
                        sqc = wkp.tile([128, TS], BF16, tag="sqc", bufs=3,
                                       name="sqc")
                        nc.scalar.activation(out=sqc, in_=srcbf[:, kc, tsl],
                                             func=AF.Square)
                        nc.tensor.matmul(
                            pmu[0:1, :], onescol_s[:], srcbf[:, kc, tsl],
                            start=(kc == 0), stop=(kc == KC - 1))
                        nc.tensor.matmul(
                            psq[0:1, :], onescol_s[:], sqc,
                            start=(kc == 0), stop=(kc == KC - 1))
                    # token-major stats for this half via PE transposes
                    stok = sp.tile([128, NJ, 2], F32, tag="stok", bufs=2,
                                   name="stok")
                    for s, pstat in ((0, pmu), (1, psq)):
                        srow = sp.tile([1, TS], F32, tag="srow", bufs=2,
                                       name="srow")
                        nc.vector.tensor_copy(out=srow, in_=pstat[0:1, :])
                        ptk = ps.tile([128, TS], F32, tag="ps", name="ptk")
                        for jj in range(NJ):
                            nc.tensor.transpose(
                                ptk[:, jj:jj + 1],
                                srow[0:1, jj * 128:(jj + 1) * 128],
                                ident_s[0:1, 0:1])
                        nc.vector.tensor_copy(out=stok[:, :, s],
                                              in_=ptk[:, 0:NJ])
                    nc.vector.tensor_scalar_mul(stok, stok, 1.0 / D)
                    mu = stok[:, :, 0]
                    m2 = stok[:, :, 1]
                    var_t = sp.tile([128, NJ], F32, tag="var_t", bufs=2,
                                    name="var_t")
                    nc.vector.tensor_tensor(var_t, mu, mu, OP.mult)
                    nc.vector.tensor_tensor(var_t, m2, var_t, OP.subtract)
                    nc.scalar.activation(out=var_t, in_=var_t, func=AF.Sqrt,
                                         bias=eps_p[:])
                    st2 = sp.tile([128, NJ, 2], F32, tag="st2", bufs=2,
                                  name="st2")
                    nc.vector.reciprocal_approx_fast(out=st2[:, :, 0],
                                                     in_=var_t)
                    nc.vector.tensor_tensor(st2[:, :, 1], mu, st2[:, :, 0],
                                            OP.mult)
                    nc.vector.tensor_scalar_mul(st2[:, :, 1], st2[:, :, 1],
                                                -1.0)
                    # back to row layout and broadcast across partitions
                    prow = ps.tile([128, TS], F32, tag="ps", name="prow")
                    nc.tensor.transpose(
                        prow[0:2 * NJ, 0:128],
                        st2.rearrange("p a b -> p (a b)"), ident_s[:])
                    rows16 = sp.tile([2 * NJ, 128], BF16, tag="rows16",
                                     bufs=2, name="rows16")
                    nc.vector.tensor_copy(out=rows16,
                                          in_=prow[0:2 * NJ, 0:128])
                    rows0 = sp.tile([1, 2 * NJ, 128], BF16, tag="rows",
                                    bufs=2, name="rows0")
                    nc.sync.dma_start(
                        out=rows0.rearrange("p a b -> p (a b)"),
                        in_=rows16[:])
                    # apply: dst = srcbf*rstd + (-mu*rstd), bf16 throughout
                    for jj in range(NJ):
                        j = tci * NJ + jj
                        tslj = slice(j * 128, (j + 1) * 128)
                        prep_r = wkp.tile([128, 128], BF16, tag="prep_r",
                                          bufs=2, name="prep_r")
                        nc.gpsimd.partition_broadcast(
                            prep_r, rows0[0:1, 2 * jj, :])
                        prep_n = wkp.tile([128, 128], BF16, tag="prep_n",
                                          bufs=2, name="prep_n")
                        nc.gpsimd.partition_broadcast(
                            prep_n, rows0[0:1, 2 * jj + 1, :])
                        tmp = wkp.tile([128, KC, 128], BF16, tag="lntmp",
                                       bufs=3, name="lntmp")
                        nc.vector.tensor_tensor(
                            tmp, srcbf[:, :, tslj],
                            prep_r[:, None, :].to_broadcast((128, KC, 128)),
                            OP.mult)
                        nc.vector.tensor_tensor(
                            dst[:, :, tslj], tmp,
                            prep_n[:, None, :].to_broadcast((128, KC, 128)),
                            OP.add)

            layernorm(X, Xbf, ALN)

            # ---- QKV projections ----
            def bias_mm(psum, bias_t, msl, tsl):
                """Start `psum` with the rank-1 bias contribution; returns the
                start flag for the following contraction matmuls."""
                if bias_t is None:
                    return True
                nc.tensor.matmul(psum, bias_t[0:1, msl], onesb_s[0:1, tsl],
                                 start=True, stop=False)
                return False

            # V weights resident; Vt ones-columns
            wv_t = pp.tile([128, KC, D], BF16, tag="wv", name="wv_t")
            nc.sync.dma_start(out=wv_t, in_=wv_d[:])
            for h in range(H):
                nc.vector.memset(Vt[:, :, h * VS + HD: h * VS + HD + 1], 1.0)

            # ---- attention ----
            # S^T[t, q] = sum_hd K^T[hd, t] Q^T[hd, q]  (K=HD contraction).
            # The HPC heads of one partition group issue their score matmuls
            # back-to-back: disjoint PE row-groups run them concurrently.
            def softmax_norm(py, h, mc, half, qsl):
                """y^T[hd,q] = py[hd,q] / py[HD,q], written to YT — via
                GpSimd broadcast + DVE approx-reciprocal; no PE involvement."""
                hsl = slice(half * HD, (half + 1) * HD)
                den = wkp.tile([1, TQ], F32, tag="den", bufs=2, name="den")
                nc.scalar.copy(out=den, in_=py[HD: HD + 1, :])
                rep = wkp.tile([HD, TQ], F32, tag="rep", bufs=3, name="rep")
                nc.gpsimd.partition_broadcast(rep, den[:])
                rrec = wkp.tile([HD, TQ], F32, tag="rrec", bufs=3, name="rrec")
                nc.vector.reciprocal_approx_fast(out=rrec, in_=rep)
                nc.vector.tensor_tensor(
                    YT[hsl, mc, qsl], py[:HD, :], rrec[:], OP.mult)

            def attn_block(pys, mc, qc, tch):
                """scores+exp+mask+PV for one (pair, qc, key-chunk)."""
                qsl = slice(qc * TQ, (qc + 1) * TQ)
                tc_lo_diag = qc * TQ // 128
                tc_hi = (qc + 1) * TQ // 128
                t128 = slice(tch * 128, (tch + 1) * 128)
                dq = max(0, tch - tc_lo_diag) * 128
                rq = slice(dq, TQ)
                qslr = slice(qc * TQ + dq, (qc + 1) * TQ)
                pscs = []
                for half in range(HPC):
                    hsl = slice(half * HD, (half + 1) * HD)
                    psc = ps.tile([128, TQ], F32, tag="ps",
                                  name=f"psc{half}")
                    nc.tensor.matmul(
                        psc[:, rq], KT[hsl, mc, t128],
                        QT[hsl, mc, qslr], start=True, stop=True)
                    pscs.append(psc)
                for half in range(HPC):
                    h = mc * HPC + half
                    pexp = wkp.tile([128, TQ], BF16, tag="pexp",
                                    bufs=4, name="pexp")
                    nc.scalar.activation(out=pexp[:, rq],
                                         in_=pscs[half][:, rq],
                                         func=AF.Exp)
                    if tch >= tc_lo_diag:
                        nc.vector.tensor_tensor(
                            pexp[:, dq:dq + 128],
                            pexp[:, dq:dq + 128], masks_s[:], OP.mult)
                    nc.tensor.matmul(
                        pys[half][:VS, rq],
                        Vt[:, tch, h * VS: (h + 1) * VS], pexp[:, rq],
                        start=(tch == 0), stop=(tch == tc_hi - 1))

            # Interleave the light lower-triangular qc=0 work into qc=1's
            # stream so the PE never starves on the softmax chains.
            assert NQ in (1, 2)
            assert H // HPC == MC  # head-pair groups == feature chunks
            NTQ = TQ // 128
            for mc in range(H // HPC):
                # QKV projections for this head pair (PE-dense work that
                # overlaps the previous pair's softmax chains)
                msl = slice(mc * 128, (mc + 1) * 128)
                for nm, wten, dstT in (("bq", wq_d, QT), ("bk", wk_d, KT)):
                    wt = wpool.tile([128, KC, 128], BF16, tag="w_qk", bufs=3,
                                    name="wt_qk")
                    nc.sync.dma_start(out=wt, in_=wten[mc])
                    for tci in range(T // TS):
                        tsl = slice(tci * TS, (tci + 1) * TS)
                        pq = ps.tile([128, TS], F32, tag="ps", name="pq")
                        st = bias_mm(pq, biases.get(nm), msl, tsl)
                        for kc in range(KC):
                            nc.tensor.matmul(
                                pq, wt[:, kc, :], ALN[:, kc, tsl],
                                start=st and (kc == 0), stop=(kc == KC - 1))
                        nc.scalar.mul(dstT[:, mc, tsl], pq[:],
                                      scale if dstT is QT else 1.0)
                # V columns for this pair, token-major
                for tch in range(NT):
                    t128 = slice(tch * 128, (tch + 1) * 128)
                    pv = ps.tile([128, TQ], F32, tag="ps", name="pv")
                    pvs = pv[:, 0:128]
                    st = True
                    if with_bias:
                        nc.tensor.matmul(pvs, onesb_s[0:1, 0:128],
                                         biases["bv"][0:1, msl],
                                         start=True, stop=False)
                        st = False
                    for kc in range(KC):
                        nc.tensor.matmul(
                            pvs, ALN[:, kc, t128], wv_t[:, kc, msl],
                            start=st and (kc == 0), stop=(kc == KC - 1))
                    dstv = Vt[:, tch, mc * HPC * VS: (mc + 1) * HPC * VS]
                    dstv = dstv.rearrange("p (h c) -> p h c", c=VS)[:, :, 0:HD]
                    nc.vector.tensor_copy(
                        out=dstv, in_=pvs.rearrange("p (h c) -> p h c", c=HD))
                # attention for this pair
                pys = {qc: [ps.tile([128, TQ], F32, tag="ps",
                                    name=f"py{qc}_{half}")
                            for half in range(HPC)]
                       for qc in range(NQ)}
                for tch in range(NQ * NTQ):
                    if NQ == 2 and tch < NTQ:
                        attn_block(pys[0], mc, 0, tch)
                    attn_block(pys[NQ - 1], mc, NQ - 1, tch)
                    if NQ == 2 and tch == NTQ - 1:
                        # qc=0 accumulators are complete: normalize now so
                        # their PSUM banks free before the next pair starts
                        for half in range(HPC):
                            softmax_norm(pys[0][half], mc * HPC + half, mc,
                                         half, slice(0, TQ))
                qc = NQ - 1
                qsl = slice(qc * TQ, (qc + 1) * TQ)
                for half in range(HPC):
                    softmax_norm(pys[qc][half], mc * HPC + half, mc,
                                 half, qsl)

            # ---- attn out-projection + residual (tci outer so X1 halves
            # complete early and LN2 statistics can overlap) ----
            for tci in range(T // TS):
                tsl = slice(tci * TS, (tci + 1) * TS)
                for mc in range(MC):
                    msl = slice(mc * 128, (mc + 1) * 128)
                    wt = wpool.tile([128, KC, 128], BF16, tag="w_p", bufs=3,
                                    name="wt_p")
                    nc.sync.dma_start(out=wt, in_=wp_d[mc])
                    po = ps.tile([128, TS], F32, tag="ps", name="po")
                    st = bias_mm(po, biases.get("bp"), msl, tsl)
                    for kc in range(KC):
                        nc.tensor.matmul(
                            po, wt[:, kc, :], YT[:, kc, tsl],
                            start=st and (kc == 0), stop=(kc == KC - 1))
                    nc.vector.tensor_tensor(
                        X1[:, mc, tsl], X[:, mc, tsl], po[:], OP.add)
                    nc.gpsimd.tensor_copy(out=X1bf[:, mc, tsl],
                                          in_=X1[:, mc, tsl])

            # ---- LN2 (A2 reuses ALN's slot; ALN is dead after QKV) ----
            A2 = pp.tile([128, KC, T], BF16, tag="ALN", name="A2")
            layernorm(X1, X1bf, A2)

            # ---- MLP: fc+gelu feeding cp accumulators, per 512-token half --
            # PSUM: MC pc accumulators held + 2 ph cycling = 8 banks exactly.
            for qc in range(T // TS):
                tsl = slice(qc * TS, (qc + 1) * TS)
                pcs = []
                for mc in range(MC):
                    pc = ps.tile([128, TS], F32, tag="ps", name=f"pc{mc}")
                    st = bias_mm(pc, biases.get("bc"),
                                 slice(mc * 128, (mc + 1) * 128), tsl)
                    pcs.append((pc, st))
                for fc in range(FC):
                    fsl = slice(fc * 128, (fc + 1) * 128)
                    wt = wpool.tile([128, KC, 128], BF16, tag="w_f", bufs=3,
                                    name="wt_f")
                    nc.sync.dma_start(out=wt, in_=wf_d[fc])
                    ph = ps.tile([128, TS], F32, tag="ps", name="ph")
                    st = bias_mm(ph, biases.get("bf"), fsl, tsl)
                    for kc in range(KC):
                        nc.tensor.matmul(
                            ph, wt[:, kc, :], A2[:, kc, tsl],
                            start=st and (kc == 0), stop=(kc == KC - 1))
                    hgel = wkp.tile([128, TS], BF16, tag="hgel", bufs=3,
                                    name="hgel")
                    nc.scalar.activation(out=hgel, in_=ph, func=gelu_func)
                    wtc = wpool.tile([128, MC, 128], BF16, tag="w_c", bufs=3,
                                     name="wt_c")
                    nc.sync.dma_start(out=wtc, in_=wc_d[fc])
                    for mc in range(MC):
                        pc, st = pcs[mc]
                        nc.tensor.matmul(
                            pc, wtc[:, mc, :], hgel,
                            start=st and (fc == 0), stop=(fc == FC - 1))
                for mc in range(MC):
                    pc, _ = pcs[mc]
                    ot = wkp.tile([128, TS], F32, tag="ot", bufs=3, name="ot")
                    nc.vector.tensor_tensor(ot, X1[:, mc, tsl], pc[:], OP.add)
                    nc.sync.dma_start(out=outT_t[:, mc, tsl], in_=ot)

    nc.finalize()
    return nc


# --------------------------------------------------------------------------
# Host-side input prep
# --------------------------------------------------------------------------
def _pack_lhsT(w):
    """[Dk, N] -> [N//128, 128, Dk//128, 128] contiguous lhsT tiles."""
    Dk, N = w.shape
    return np.ascontiguousarray(
        w.reshape(Dk // 128, 128, N // 128, 128).transpose(2, 1, 0, 3))


def prepare_weights(wq, bq, wk, bk, wv, bv, w_proj, b_proj, g1, be1, g2, be2,
                    w_fc, b_fc, w_cp, b_cp):
    """Fold LN affines + reshape heads; return packed bf16 arrays."""
    bf = ml_dtypes.bfloat16
    H_, D_, HD_ = wq.shape
    # [H, D, HD] -> [D, H*HD]
    wq2 = wq.transpose(1, 0, 2).reshape(D_, H_ * HD_).astype(np.float64)
    wk2 = wk.transpose(1, 0, 2).reshape(D_, H_ * HD_).astype(np.float64)
    wv2 = wv.transpose(1, 0, 2).reshape(D_, H_ * HD_).astype(np.float64)
    g1 = g1.astype(np.float64); be1 = be1.astype(np.float64)
    g2 = g2.astype(np.float64); be2 = be2.astype(np.float64)
    w_fc64 = w_fc.astype(np.float64)
    # fold LN affine: LN_aff(x) = n(x)*g + be  =>  W' = g[:,None]*W,
    # b' = b + be @ W
    arrs = {
        "wq": _pack_lhsT((g1[:, None] * wq2).astype(bf)),
        "wk": _pack_lhsT((g1[:, None] * wk2).astype(bf)),
        "wv": np.ascontiguousarray(
            (g1[:, None] * wv2).astype(bf)
            .reshape(-1, 128, wv2.shape[1]).transpose(1, 0, 2)),
        "wp": _pack_lhsT(w_proj.astype(bf)),
        "wf": _pack_lhsT((g2[:, None] * w_fc64).astype(bf)),
        "wc": np.ascontiguousarray(
            w_cp.astype(bf).reshape(-1, 128, w_cp.shape[1] // 128, 128)),
    }
    bias_arrs = {
        "bq": bq.reshape(-1).astype(np.float64) + be1 @ wq2,
        "bk": bk.reshape(-1).astype(np.float64) + be1 @ wk2,
        "bv": bv.reshape(-1).astype(np.float64) + be1 @ wv2,
        "bp": b_proj.astype(np.float64),
        "bf": b_fc.astype(np.float64) + be2 @ w_fc64,
        "bc": b_cp.astype(np.float64),
    }
    any_bias = bool(any(np.any(v != 0) for v in bias_arrs.values()))
    if any_bias:
        for k, v in bias_arrs.items():
            arrs[k] = v.astype(bf).reshape(1, -1)
    return arrs, any_bias


_NC_CACHE = {}


def kernel(**inputs):
    x = np.asarray(inputs["x"], np.float32)
    arrs, any_bias = prepare_weights(
        *(np.asarray(inputs[k]) for k in (
            "wq", "bq", "wk", "bk", "wv", "bv", "w_proj", "b_proj",
            "g1", "be1", "g2", "be2", "w_fc", "b_fc", "w_cp", "b_cp")))
    key = ("full", any_bias)
    if key not in _NC_CACHE:
        _NC_CACHE[key] = build_decoder_nc(with_bias=any_bias)
    nc = _NC_CACHE[key]

    in_maps = []
    for b in range(N_CORES):
        m = dict(arrs)
        m["xT"] = np.ascontiguousarray(x[b].T)
        in_maps.append(m)

    from concourse.bass_utils import run_bass_kernel_spmd
    res = run_bass_kernel_spmd(nc, in_maps, list(range(N_CORES)))
    out = np.stack([res.results[i]["outT"].T for i in range(N_CORES)])
    return out.astype(np.float32)


# revision 18
# speedup vs baseline: 1.1652x; 1.1003x over previous
"""Trainium2 Bass kernel for an nn.DecoderBlock (pre-LN GPT block).

Reference computation (per batch element, fp32):
    h  = LN(x; g1,be1);  q,k,v = per-head projections of h
    y  = causal-softmax(q k^T / sqrt(hd)) v ;  x1 = x + y @ w_proj + b_proj
    h2 = LN(x1; g2,be2); out = x1 + gelu_tanh(h2 @ w_fc + b_fc) @ w_cp + b_cp

Shapes: B=8, T=1024, D=768, H=12, HD=64, F=3072.

Strategy: pure data parallelism — batch element b runs on core b (B == n_cores
== 8); the decoder block is independent per batch element so no collectives are
needed.  On-chip, all activations are kept *feature-major* ([D, T]: features on
partitions, tokens on the free axis) so chained matmuls need no transposes:
    out^T[n, t] = sum_d W[d, n] * A^T[d, t]   (lhsT = W as stored, rhs = A^T)
Attention scores are computed transposed (S^T[t, q]) so the softmax-weighted
probabilities land directly in the [t, q] layout the P@V matmul needs as its
moving operand; the two heads sharing a 128-partition group issue their K=64
score matmuls back-to-back so the PE runs them concurrently in disjoint
row-groups.  The softmax denominator comes from augmenting V with a
ones-column (row HD of the PV output is sum_t P[t,q]).  Softmax max-subtraction
is skipped: post-LN scores are O(5) so fp32 exp cannot overflow.

The PE instruction stream in attention is pure matmuls: softmax normalization
runs entirely on GpSimd (partition_broadcast of the denominator row) + DVE
(reciprocal_approx_fast, multiply), so the PE never stalls on it.  LayerNorm
statistics are per-token sums gathered with ones-column matmuls; the per-token
scalar math runs 128-lane in a token-major layout reached via PE transposes,
and the results are broadcast across partitions by GpSimd.

Host-side prep (numpy): transpose x per core, fold LN affine (g,be) and all
biases into the weight matrices, pre-pack weights into DMA-contiguous tiles,
cast to bf16. Matmuls run in bf16 with fp32 PSUM accumulation; LN stats,
residuals and softmax denominators stay fp32.
"""

import numpy as np
import ml_dtypes

import concourse.bass as bass
import concourse.mybir as mybir
import concourse.tile as tile
from concourse import bacc

BF16 = mybir.dt.bfloat16
F32 = mybir.dt.float32
AF = mybir.ActivationFunctionType
OP = mybir.AluOpType

# Full-problem dimensions (hardcoded; harness contract).
B, T, D, H = 8, 1024, 768, 12
HD = D // H
F = 4 * D
EPS = 1e-5
N_CORES = 8


# --------------------------------------------------------------------------
# Bass program builder (parameterized so a small variant can be simulated)
# --------------------------------------------------------------------------
def build_decoder_nc(T=T, D=D, H=H, F=F, TQ=512, with_bias=False, eps=EPS,
                     gelu_func=AF.Gelu_apprx_tanh):
    """Build the single-core Bass program (same program runs SPMD on all cores).

    DRAM I/O layouts (all prepared host-side):
      xT    [D, T]             f32   x^T (feature-major)
      wq,wk [MC,128,KC,128]    bf16  packed lhsT tiles (LN1 affine folded in)
      wv    [128,KC,D]         bf16  rhs layout for token-major V
      wp    [MC,128,KC,128]    bf16  w_proj packed
      wf    [FC,128,KC,128]    bf16  w_fc packed (LN2 affine folded in)
      wc    [FC,128,MC,128]    bf16  w_cp packed fc-major (plain reshape)
      *_b   [1, N]             bf16  folded bias rows (only if with_bias)
      outT  [D, T]             f32   output^T
    """
    assert D % 128 == 0 and F % 128 == 0 and T % TQ == 0 and TQ % 128 == 0
    TS = min(512, T)           # token chunk for projections/LN stats
    assert T % TS == 0
    KC = D // 128          # contraction chunks over D
    FC = F // 128          # chunks over MLP hidden
    MC = D // 128          # output-feature chunks over D
    NT = T // 128          # key/token chunks of 128
    NQ = T // TQ           # query chunks of TQ
    ND = TQ // 128         # diagonal mask variants
    HPC = 128 // HD        # heads per 128-partition group (2 for HD=64)
    VS = HD + 1            # V columns per head incl. ones-column
    scale = 1.0 / np.sqrt(HD)
    assert H % HPC == 0 and 2 * NT <= 128

    nc = bacc.Bacc()

    # ---- DRAM I/O ----
    xT = nc.dram_tensor("xT", [D, T], F32, kind="ExternalInput")
    wq_d = nc.dram_tensor("wq", [MC, 128, KC, 128], BF16, kind="ExternalInput")
    wk_d = nc.dram_tensor("wk", [MC, 128, KC, 128], BF16, kind="ExternalInput")
    wv_d = nc.dram_tensor("wv", [128, KC, D], BF16, kind="ExternalInput")
    wp_d = nc.dram_tensor("wp", [MC, 128, KC, 128], BF16, kind="ExternalInput")
    wf_d = nc.dram_tensor("wf", [FC, 128, KC, 128], BF16, kind="ExternalInput")
    wc_d = nc.dram_tensor("wc", [FC, 128, MC, 128], BF16, kind="ExternalInput")
    bias_d = {}
    if with_bias:
        for nm, width in (("bq", D), ("bk", D), ("bv", D), ("bp", D),
                          ("bf", F), ("bc", D)):
            bias_d[nm] = nc.dram_tensor(nm, [1, width], BF16,
                                        kind="ExternalInput")
    outT = nc.dram_tensor("outT", [D, T], F32, kind="ExternalOutput")
    outT_t = outT[:].rearrange("(o p) t -> p o t", p=128)

    # ---- constants (embedded in the NEFF) ----
    ones_bf = nc.inline_tensor(np.ones((1, T), ml_dtypes.bfloat16), "ones_bf")
    onescol = nc.inline_tensor(np.ones((128, 1), ml_dtypes.bfloat16),
                               "onescol")
    ident_np = np.eye(128, dtype=np.float32)
    ident_c = nc.inline_tensor(ident_np, "ident_c")
    # triangular mask for the diagonal 128x128 score blocks: 1 if i <= j
    m_np = (np.arange(128)[:, None] <= np.arange(128)[None, :]).astype(
        ml_dtypes.bfloat16)
    masks_d = nc.inline_tensor(m_np, "masks")

    with tile.TileContext(nc) as tc:
        with (
            tc.tile_pool(name="persist", bufs=1) as pp,
            tc.tile_pool(name="wts", bufs=3) as wpool,
            tc.tile_pool(name="work", bufs=3) as wkp,
            tc.tile_pool(name="small", bufs=1) as sp,
            tc.tile_pool(name="ps", bufs=8, space="PSUM") as ps,
        ):
            # ---- persistent SBUF tensors ----
            X = pp.tile([128, KC, T], F32, tag="X", name="X")
            ALN = pp.tile([128, KC, T], BF16, tag="ALN", name="ALN")
            QT = pp.tile([128, KC, T], BF16, tag="QT", name="QT")
            KT = pp.tile([128, KC, T], BF16, tag="KT", name="KT")
            Vt = pp.tile([128, NT, H * VS], BF16, tag="Vt", name="Vt")
            YT = pp.tile([128, KC, T], BF16, tag="YT", name="YT")
            X1 = pp.tile([128, KC, T], F32, tag="X1", name="X1")

            onesb_s = None
            if with_bias:
                onesb_s = pp.tile([1, T], BF16, tag="onesb", name="onesb_s")
                nc.sync.dma_start(out=onesb_s, in_=ones_bf[:])
            onescol_s = pp.tile([128, 1], BF16, tag="onescol",
                                name="onescol_s")
            nc.sync.dma_start(out=onescol_s, in_=onescol[:])
            ident_s = pp.tile([128, 128], F32, tag="ident", name="ident_s")
            nc.sync.dma_start(out=ident_s, in_=ident_c[:])
            eps_p = pp.tile([128, 1], F32, tag="eps", name="eps_p")
            nc.vector.memset(eps_p, eps)
            masks_s = pp.tile([128, 128], BF16, tag="masks", name="masks_s")
            nc.sync.dma_start(out=masks_s, in_=masks_d[:])
            biases = {}
            for nm, dten in bias_d.items():
                bt = pp.tile(list(dten.shape), BF16, tag=nm, name=f"{nm}_s")
                nc.sync.dma_start(out=bt, in_=dten[:])
                biases[nm] = bt

            Xbf = pp.tile([128, KC, T], BF16, tag="Xbf", name="Xbf")
            X1bf = pp.tile([128, KC, T], BF16, tag="X1bf", name="X1bf")

            # ---- load x^T ----
            xT_t = xT[:].rearrange("(o p) t -> p o t", p=128)
            for kc in range(KC):
                nc.sync.dma_start(out=X[:, kc, :], in_=xT_t[:, kc, :])
                nc.gpsimd.tensor_copy(out=Xbf[:, kc, :], in_=X[:, kc, :])

            # ---- LayerNorm: dst = (src - mu) * rstd, cast bf16 ----
            # Per-token sums via ones-column matmuls; scalar math runs
            # 128-lane in token-major layout (PE transpose there and back);
            # GpSimd broadcasts the per-token factors across partitions.
            def layernorm(src, srcbf, dst):
                NJ = TS // 128
                for tci in range(T // TS):
                    tsl = slice(tci * TS, (tci + 1) * TS)
                    pmu = ps.tile([128, TS], F32, tag="ps", name="pmu")
                    psq = ps.tile([128, TS], F32, tag="ps", name="psq")
                    for kc in range(KC):
                        sqc = wkp.tile([128, TS], BF16, tag="sqc", bufs=3,
                                       name="sqc")
                        nc.scalar.activation(out=sqc, in_=srcbf[:, kc, tsl],
                                             func=AF.Square)
                        nc.tensor.matmul(
                            pmu[0:1, :], onescol_s[:], srcbf[:, kc, tsl],
                            start=(kc == 0), stop=(kc == KC - 1))
                        nc.tensor.matmul(
                            psq[0:1, :], onescol_s[:], sqc,
                            start=(kc == 0), stop=(kc == KC - 1))
                    # token-major stats for this half via PE transposes
                    stok = sp.tile([128, NJ, 2], F32, tag="stok", bufs=2,
                                   name="stok")
                    for s, pstat in ((0, pmu), (1, psq)):
                        srow = sp.tile([1, TS], F32, tag="srow", bufs=2,
                                       name="srow")
                        nc.vector.tensor_copy(out=srow, in_=pstat[0:1, :])
                        ptk = ps.tile([128, TS], F32, tag="ps", name="ptk")
                        for jj in range(NJ):
                            nc.tensor.transpose(
                                ptk[:, jj:jj + 1],
                                srow[0:1, jj * 128:(jj + 1) * 128],
                                ident_s[0:1, 0:1])
                        nc.vector.tensor_copy(out=stok[:, :, s],
                                              in_=ptk[:, 0:NJ])
                    nc.vector.tensor_scalar_mul(stok, stok, 1.0 / D)
                    mu = stok[:, :, 0]
                    m2 = stok[:, :, 1]
                    var_t = sp.tile([128, NJ], F32, tag="var_t", bufs=2,
                                    name="var_t")
                    nc.vector.tensor_tensor(var_t, mu, mu, OP.mult)
                    nc.vector.tensor_tensor(var_t, m2, var_t, OP.subtract)
                    nc.scalar.activation(out=var_t, in_=var_t, func=AF.Sqrt,
                                         bias=eps_p[:])
                    st2 = sp.tile([128, NJ, 2], F32, tag="st2", bufs=2,
                                  name="st2")
                    nc.vector.reciprocal_approx_fast(out=st2[:, :, 0],
                                                     in_=var_t)
                    nc.vector.tensor_tensor(st2[:, :, 1], mu, st2[:, :, 0],
                                            OP.mult)
                    nc.vector.tensor_scalar_mul(st2[:, :, 1], st2[:, :, 1],
                                                -1.0)
                    # back to row layout and broadcast across partitions
                    prow = ps.tile([128, TS], F32, tag="ps", name="prow")
                    nc.tensor.transpose(
                        prow[0:2 * NJ, 0:128],
                        st2.rearrange("p a b -> p (a b)"), ident_s[:])
                    rows16 = sp.tile([2 * NJ, 128], BF16, tag="rows16",
                                     bufs=2, name="rows16")
                    nc.vector.tensor_copy(out=rows16,
                                          in_=prow[0:2 * NJ, 0:128])
                    rows0 = sp.tile([1, 2 * NJ, 128], BF16, tag="rows",
                                    bufs=2, name="rows0")
                    nc.sync.dma_start(
                        out=rows0.rearrange("p a b -> p (a b)"),
                        in_=rows16[:])
                    # apply: dst = srcbf*rstd + (-mu*rstd), bf16 throughout
                    for jj in range(NJ):
                        j = tci * NJ + jj
                        tslj = slice(j * 128, (j + 1) * 128)
                        prep_r = wkp.tile([128, 128], BF16, tag="prep_r",
                                          bufs=2, name="prep_r")
                        nc.gpsimd.partition_broadcast(
                            prep_r, rows0[0:1, 2 * jj, :])
                        prep_n = wkp.tile([128, 128], BF16, tag="prep_n",
                                          bufs=2, name="prep_n")
                        nc.gpsimd.partition_broadcast(
                            prep_n, rows0[0:1, 2 * jj + 1, :])
                        tmp = wkp.tile([128, KC, 128], BF16, tag="lntmp",
                                       bufs=3, name="lntmp")
                        nc.vector.tensor_tensor(
                            tmp, srcbf[:, :, tslj],
                            prep_r[:, None, :].to_broadcast((128, KC, 128)),
                            OP.mult)
                        nc.vector.tensor_tensor(
                            dst[:, :, tslj], tmp,
                            prep_n[:, None, :].to_broadcast((128, KC, 128)),
                            OP.add)

            layernorm(X, Xbf, ALN)

            # ---- QKV projections ----
            def bias_mm(psum, bias_t, msl, tsl):
                """Start `psum` with the rank-1 bias contribution; returns the
                start flag for the following contraction matmuls."""
                if bias_t is None:
                    return True
                nc.tensor.matmul(psum, bias_t[0:1, msl], onesb_s[0:1, tsl],
                                 start=True, stop=False)
                return False

            # V weights resident; Vt ones-columns
            wv_t = pp.tile([128, KC, D], BF16, tag="wv", name="wv_t")
            nc.sync.dma_start(out=wv_t, in_=wv_d[:])
            for h in range(H):
                nc.vector.memset(Vt[:, :, h * VS + HD: h * VS + HD + 1], 1.0)

            # ---- attention (software-pipelined with next pair's QKV) --
            # S^T[t, q] = sum_hd K^T[hd, t] Q^T[hd, q]  (K=HD contraction).
            # The PE stream is in-order, so PV matmuls that wait on the ACT
            # exp would stall everything behind them.  To keep the PE dense,
            # the NEXT head-pair's QKV/V matmuls (independent: they read only
            # ALN) are emitted as "filler units" interleaved between this
            # pair's score and PV matmuls.
            def softmax_norm(py, h, mc, half, qsl):
                """y^T[hd,q] = py[hd,q] / py[HD,q], written to YT — via
                GpSimd broadcast + DVE approx-reciprocal; no PE involvement."""
                hsl = slice(half * HD, (half + 1) * HD)
                den = wkp.tile([1, TQ], F32, tag="den", bufs=2, name="den")
                nc.vector.tensor_copy(out=den, in_=py[HD: HD + 1, :])
                rep = wkp.tile([HD, TQ], F32, tag="rep", bufs=3, name="rep")
                nc.gpsimd.partition_broadcast(rep, den[:])
                rrec = wkp.tile([HD, TQ], F32, tag="rrec", bufs=3, name="rrec")
                nc.vector.reciprocal_approx_fast(out=rrec, in_=rep)
                nc.vector.tensor_tensor(
                    YT[hsl, mc, qsl], py[:HD, :], rrec[:], OP.mult)

            def attn_scores(pys, mc, qc, tch):
                """score matmuls + exp + mask for one (pair, qc, key chunk);
                returns a closure emitting the matching PV matmuls."""
                tc_lo_diag = qc * TQ // 128
                tc_hi = (qc + 1) * TQ // 128
                t128 = slice(tch * 128, (tch + 1) * 128)
                dq = max(0, tch - tc_lo_diag) * 128
                rq = slice(dq, TQ)
                qslr = slice(qc * TQ + dq, (qc + 1) * TQ)
                pexps = []
                for half in range(HPC):
                    hsl = slice(half * HD, (half + 1) * HD)
                    psc = ps.tile([128, TQ], F32, tag="ps",
                                  name=f"psc{half}")
                    nc.tensor.matmul(
                        psc[:, rq], KT[hsl, mc, t128],
                        QT[hsl, mc, qslr], start=True, stop=True)
                    pexp = wkp.tile([128, TQ], BF16, tag="pexp",
                                    bufs=4, name="pexp")
                    nc.scalar.activation(out=pexp[:, rq], in_=psc[:, rq],
                                         func=AF.Exp)
                    if tch >= tc_lo_diag:
                        nc.vector.tensor_tensor(
                            pexp[:, dq:dq + 128],
                            pexp[:, dq:dq + 128], masks_s[:], OP.mult)
                    pexps.append(pexp)

                def emit_pv():
                    for half in range(HPC):
                        h = mc * HPC + half
                        nc.tensor.matmul(
                            pys[half][:VS, rq],
                            Vt[:, tch, h * VS: (h + 1) * VS],
                            pexps[half][:, rq],
                            start=(tch == 0), stop=(tch == tc_hi - 1))
                return emit_pv

            def make_filler(mc):
                """Filler units (closures) for pair mc's QKV + V matmuls."""
                msl = slice(mc * 128, (mc + 1) * 128)
                units = []
                for nm, wten, dstT in (("bq", wq_d, QT), ("bk", wk_d, KT)):
                    wt = wpool.tile([128, KC, 128], BF16, tag="w_qk", bufs=3,
                                    name="wt_qk")
                    nc.sync.dma_start(out=wt, in_=wten[mc])
                    for tci in range(T // TS):
                        def qkv_unit(nm=nm, wt=wt, dstT=dstT, tci=tci):
                            tsl = slice(tci * TS, (tci + 1) * TS)
                            pq = ps.tile([128, TS], F32, tag="ps", name="pq")
                            st = bias_mm(pq, biases.get(nm), msl, tsl)
                            for kc in range(KC):
                                nc.tensor.matmul(
                                    pq, wt[:, kc, :], ALN[:, kc, tsl],
                                    start=st and (kc == 0),
                                    stop=(kc == KC - 1))
                            nc.scalar.mul(dstT[:, mc, tsl], pq[:],
                                          scale if dstT is QT else 1.0)
                        units.append(qkv_unit)
                for tch in range(NT):
                    def v_unit(tch=tch):
                        t128 = slice(tch * 128, (tch + 1) * 128)
                        pv = ps.tile([128, TQ], F32, tag="ps", name="pv")
                        pvs = pv[:, 0:128]
                        st = True
                        if with_bias:
                            nc.tensor.matmul(pvs, onesb_s[0:1, 0:128],
                                             biases["bv"][0:1, msl],
                                             start=True, stop=False)
                            st = False
                        for kc in range(KC):
                            nc.tensor.matmul(
                                pvs, ALN[:, kc, t128], wv_t[:, kc, msl],
                                start=st and (kc == 0), stop=(kc == KC - 1))
                        dstv = Vt[:, tch, mc * HPC * VS: (mc + 1) * HPC * VS]
                        dstv = dstv.rearrange("p (h c) -> p h c",
                                              c=VS)[:, :, 0:HD]
                        nc.vector.tensor_copy(
                            out=dstv,
                            in_=pvs.rearrange("p (h c) -> p h c", c=HD))
                    units.append(v_unit)
                return units

            # V weights resident; Vt ones-columns
            wv_t = pp.tile([128, KC, D], BF16, tag="wv", name="wv_t")
            nc.sync.dma_start(out=wv_t, in_=wv_d[:])
            for h in range(H):
                nc.vector.memset(Vt[:, :, h * VS + HD: h * VS + HD + 1], 1.0)

            assert NQ in (1, 2)
            assert H // HPC == MC  # head-pair groups == feature chunks
            NTQ = TQ // 128
            NPAIR = H // HPC
            for u in make_filler(0):   # prologue: first pair's QKV/V
                u()
            for mc in range(NPAIR):
                filler = make_filler(mc + 1) if mc + 1 < NPAIR else []
                fi = 0
                pys = {qc: [ps.tile([128, TQ], F32, tag="ps",
                                    name=f"py{qc}_{half}")
                            for half in range(HPC)]
                       for qc in range(NQ)}
                for tch in range(NQ * NTQ):
                    pvs_cbs = []
                    if NQ == 2 and tch < NTQ:
                        pvs_cbs.append(attn_scores(pys[0], mc, 0, tch))
                    pvs_cbs.append(attn_scores(pys[NQ - 1], mc, NQ - 1, tch))
                    # independent PE work gives the exps time to finish
                    if fi < len(filler):
                        filler[fi](); fi += 1
                    for cb in pvs_cbs:
                        cb()
                    if NQ == 2 and tch == NTQ - 1:
                        for half in range(HPC):
                            softmax_norm(pys[0][half], mc * HPC + half, mc,
                                         half, slice(0, TQ))
                while fi < len(filler):
                    filler[fi](); fi += 1
                qc = NQ - 1
                qsl = slice(qc * TQ, (qc + 1) * TQ)
                for half in range(HPC):
                    softmax_norm(pys[qc][half], mc * HPC + half, mc,
                                 half, qsl)

            # ---- attn out-projection + residual (tci outer so X1 halves
            # complete early and LN2 statistics can overlap) ----
            for tci in range(T // TS):
                tsl = slice(tci * TS, (tci + 1) * TS)
                for mc in range(MC):
                    msl = slice(mc * 128, (mc + 1) * 128)
                    wt = wpool.tile([128, KC, 128], BF16, tag="w_p", bufs=3,
                                    name="wt_p")
                    nc.sync.dma_start(out=wt, in_=wp_d[mc])
                    po = ps.tile([128, TS], F32, tag="ps", name="po")
                    st = bias_mm(po, biases.get("bp"), msl, tsl)
                    for kc in range(KC):
                        nc.tensor.matmul(
                            po, wt[:, kc, :], YT[:, kc, tsl],
                            start=st and (kc == 0), stop=(kc == KC - 1))
                    nc.vector.tensor_tensor(
                        X1[:, mc, tsl], X[:, mc, tsl], po[:], OP.add)
                    nc.gpsimd.tensor_copy(out=X1bf[:, mc, tsl],
                                          in_=X1[:, mc, tsl])

            # ---- LN2 (A2 reuses ALN's slot; ALN is dead after QKV) ----
            A2 = pp.tile([128, KC, T], BF16, tag="ALN", name="A2")
            layernorm(X1, X1bf, A2)

            # ---- MLP: fc+gelu feeding cp accumulators, per 512-token half --
            # PSUM: MC pc accumulators held + 2 ph cycling = 8 banks exactly.
            for qc in range(T // TS):
                tsl = slice(qc * TS, (qc + 1) * TS)
                pcs = []
                for mc in range(MC):
                    pc = ps.tile([128, TS], F32, tag="ps", name=f"pc{mc}")
                    st = bias_mm(pc, biases.get("bc"),
                                 slice(mc * 128, (mc + 1) * 128), tsl)
                    pcs.append((pc, st))
                for fc in range(FC):
                    fsl = slice(fc * 128, (fc + 1) * 128)
                    wt = wpool.tile([128, KC, 128], BF16, tag="w_f", bufs=3,
                                    name="wt_f")
                    nc.sync.dma_start(out=wt, in_=wf_d[fc])
                    ph = ps.tile([128, TS], F32, tag="ps", name="ph")
                    st = bias_mm(ph, biases.get("bf"), fsl, tsl)
                    for kc in range(KC):
                        nc.tensor.matmul(
                            ph, wt[:, kc, :], A2[:, kc, tsl],
                            start=st and (kc == 0), stop=(kc == KC - 1))
                    hgel = wkp.tile([128, TS], BF16, tag="hgel", bufs=3,
                                    name="hgel")
                    nc.scalar.activation(out=hgel, in_=ph, func=gelu_func)
                    wtc = wpool.tile([128, MC, 128], BF16, tag="w_c", bufs=3,
                                     name="wt_c")
                    nc.sync.dma_start(out=wtc, in_=wc_d[fc])
                    for mc in range(MC):
                        pc, st = pcs[mc]
                        nc.tensor.matmul(
                            pc, wtc[:, mc, :], hgel,
                            start=st and (fc == 0), stop=(fc == FC - 1))
                for mc in range(MC):
                    pc, _ = pcs[mc]
                    ot = wkp.tile([128, TS], F32, tag="ot", bufs=3, name="ot")
                    nc.vector.tensor_tensor(ot, X1[:, mc, tsl], pc[:], OP.add)
                    nc.sync.dma_start(out=outT_t[:, mc, tsl], in_=ot)

    nc.finalize()
    return nc


# --------------------------------------------------------------------------
# Host-side input prep
# --------------------------------------------------------------------------
def _pack_lhsT(w):
    """[Dk, N] -> [N//128, 128, Dk//128, 128] contiguous lhsT tiles."""
    Dk, N = w.shape
    return np.ascontiguousarray(
        w.reshape(Dk // 128, 128, N // 128, 128).transpose(2, 1, 0, 3))


def prepare_weights(wq, bq, wk, bk, wv, bv, w_proj, b_proj, g1, be1, g2, be2,
                    w_fc, b_fc, w_cp, b_cp):
    """Fold LN affines + reshape heads; return packed bf16 arrays."""
    bf = ml_dtypes.bfloat16
    H_, D_, HD_ = wq.shape
    # [H, D, HD] -> [D, H*HD]
    wq2 = wq.transpose(1, 0, 2).reshape(D_, H_ * HD_).astype(np.float64)
    wk2 = wk.transpose(1, 0, 2).reshape(D_, H_ * HD_).astype(np.float64)
    wv2 = wv.transpose(1, 0, 2).reshape(D_, H_ * HD_).astype(np.float64)
    g1 = g1.astype(np.float64); be1 = be1.astype(np.float64)
    g2 = g2.astype(np.float64); be2 = be2.astype(np.float64)
    w_fc64 = w_fc.astype(np.float64)
    # fold LN affine: LN_aff(x) = n(x)*g + be  =>  W' = g[:,None]*W,
    # b' = b + be @ W
    arrs = {
        "wq": _pack_lhsT((g1[:, None] * wq2).astype(bf)),
        "wk": _pack_lhsT((g1[:, None] * wk2).astype(bf)),
        "wv": np.ascontiguousarray(
            (g1[:, None] * wv2).astype(bf)
            .reshape(-1, 128, wv2.shape[1]).transpose(1, 0, 2)),
        "wp": _pack_lhsT(w_proj.astype(bf)),
        "wf": _pack_lhsT((g2[:, None] * w_fc64).astype(bf)),
        "wc": np.ascontiguousarray(
            w_cp.astype(bf).reshape(-1, 128, w_cp.shape[1] // 128, 128)),
    }
    bias_arrs = {
        "bq": bq.reshape(-1).astype(np.float64) + be1 @ wq2,
        "bk": bk.reshape(-1).astype(np.float64) + be1 @ wk2,
        "bv": bv.reshape(-1).astype(np.float64) + be1 @ wv2,
        "bp": b_proj.astype(np.float64),
        "bf": b_fc.astype(np.float64) + be2 @ w_fc64,
        "bc": b_cp.astype(np.float64),
    }
    any_bias = bool(any(np.any(v != 0) for v in bias_arrs.values()))
    if any_bias:
        for k, v in bias_arrs.items():
            arrs[k] = v.astype(bf).reshape(1, -1)
    return arrs, any_bias


_NC_CACHE = {}


def kernel(**inputs):
    x = np.asarray(inputs["x"], np.float32)
    arrs, any_bias = prepare_weights(
        *(np.asarray(inputs[k]) for k in (
            "wq", "bq", "wk", "bk", "wv", "bv", "w_proj", "b_proj",
            "g1", "be1", "g2", "be2", "w_fc", "b_fc", "w_cp", "b_cp")))
    key = ("full", any_bias)
    if key not in _NC_CACHE:
        _NC_CACHE[key] = build_decoder_nc(with_bias=any_bias)
    nc = _NC_CACHE[key]

    in_maps = []
    for b in range(N_CORES):
        m = dict(arrs)
        m["xT"] = np.ascontiguousarray(x[b].T)
        in_maps.append(m)

    from concourse.bass_utils import run_bass_kernel_spmd
    res = run_bass_kernel_spmd(nc, in_maps, list(range(N_CORES)))
    out = np.stack([res.results[i]["outT"].T for i in range(N_CORES)])
    return out.astype(np.float32)


# revision 19
# speedup vs baseline: 1.1911x; 1.0223x over previous
"""Trainium2 Bass kernel for an nn.DecoderBlock (pre-LN GPT block).

Reference computation (per batch element, fp32):
    h  = LN(x; g1,be1);  q,k,v = per-head projections of h
    y  = causal-softmax(q k^T / sqrt(hd)) v ;  x1 = x + y @ w_proj + b_proj
    h2 = LN(x1; g2,be2); out = x1 + gelu_tanh(h2 @ w_fc + b_fc) @ w_cp + b_cp

Shapes: B=8, T=1024, D=768, H=12, HD=64, F=3072.

Strategy: pure data parallelism — batch element b runs on core b (B == n_cores
== 8); the decoder block is independent per batch element so no collectives are
needed.  On-chip, all activations are kept *feature-major* ([D, T]: features on
partitions, tokens on the free axis) so chained matmuls need no transposes:
    out^T[n, t] = sum_d W[d, n] * A^T[d, t]   (lhsT = W as stored, rhs = A^T)
Attention scores are computed transposed (S^T[t, q]) so the softmax-weighted
probabilities land directly in the [t, q] layout the P@V matmul needs as its
moving operand; the two heads sharing a 128-partition group issue their K=64
score matmuls back-to-back so the PE runs them concurrently in disjoint
row-groups.  The softmax denominator comes from augmenting V with a
ones-column (row HD of the PV output is sum_t P[t,q]).  Softmax max-subtraction
is skipped: post-LN scores are O(5) so fp32 exp cannot overflow.

The PE instruction stream in attention is pure matmuls: softmax normalization
runs entirely on GpSimd (partition_broadcast of the denominator row) + DVE
(reciprocal_approx_fast, multiply), so the PE never stalls on it.  LayerNorm
statistics are per-token sums gathered with ones-column matmuls; the per-token
scalar math runs 128-lane in a token-major layout reached via PE transposes,
and the results are broadcast across partitions by GpSimd.

Host-side prep (numpy): transpose x per core, fold LN affine (g,be) and all
biases into the weight matrices, pre-pack weights into DMA-contiguous tiles,
cast to bf16. Matmuls run in bf16 with fp32 PSUM accumulation; LN stats,
residuals and softmax denominators stay fp32.
"""

import numpy as np
import ml_dtypes

import concourse.bass as bass
import concourse.mybir as mybir
import concourse.tile as tile
from concourse import bacc

BF16 = mybir.dt.bfloat16
F32 = mybir.dt.float32
AF = mybir.ActivationFunctionType
OP = mybir.AluOpType

# Full-problem dimensions (hardcoded; harness contract).
B, T, D, H = 8, 1024, 768, 12
HD = D // H
F = 4 * D
EPS = 1e-5
N_CORES = 8


# --------------------------------------------------------------------------
# Bass program builder (parameterized so a small variant can be simulated)
# --------------------------------------------------------------------------
def build_decoder_nc(T=T, D=D, H=H, F=F, TQ=512, with_bias=False, eps=EPS,
                     gelu_func=AF.Gelu_apprx_tanh):
    """Build the single-core Bass program (same program runs SPMD on all cores).

    DRAM I/O layouts (all prepared host-side):
      xT    [D, T]             f32   x^T (feature-major)
      wq,wk [MC,128,KC,128]    bf16  packed lhsT tiles (LN1 affine folded in)
      wv    [128,KC,D]         bf16  rhs layout for token-major V
      wp    [MC,128,KC,128]    bf16  w_proj packed
      wf    [FC,128,KC,128]    bf16  w_fc packed (LN2 affine folded in)
      wc    [FC,128,MC,128]    bf16  w_cp packed fc-major (plain reshape)
      *_b   [1, N]             bf16  folded bias rows (only if with_bias)
      outT  [D, T]             f32   output^T
    """
    assert D % 128 == 0 and F % 128 == 0 and T % TQ == 0 and TQ % 128 == 0
    TS = min(512, T)           # token chunk for projections/LN stats
    assert T % TS == 0
    KC = D // 128          # contraction chunks over D
    FC = F // 128          # chunks over MLP hidden
    MC = D // 128          # output-feature chunks over D
    NT = T // 128          # key/token chunks of 128
    NQ = T // TQ           # query chunks of TQ
    ND = TQ // 128         # diagonal mask variants
    HPC = 128 // HD        # heads per 128-partition group (2 for HD=64)
    VS = HD + 1            # V columns per head incl. ones-column
    scale = 1.0 / np.sqrt(HD)
    assert H % HPC == 0 and 2 * NT <= 128

    nc = bacc.Bacc()

    # ---- DRAM I/O ----
    xT = nc.dram_tensor("xT", [D, T], F32, kind="ExternalInput")
    wq_d = nc.dram_tensor("wq", [MC, 128, KC, 128], BF16, kind="ExternalInput")
    wk_d = nc.dram_tensor("wk", [MC, 128, KC, 128], BF16, kind="ExternalInput")
    wv_d = nc.dram_tensor("wv", [128, KC, D], BF16, kind="ExternalInput")
    wp_d = nc.dram_tensor("wp", [MC, 128, KC, 128], BF16, kind="ExternalInput")
    wf_d = nc.dram_tensor("wf", [FC, 128, KC, 128], BF16, kind="ExternalInput")
    wc_d = nc.dram_tensor("wc", [FC, 128, MC, 128], BF16, kind="ExternalInput")
    bias_d = {}
    if with_bias:
        for nm, width in (("bq", D), ("bk", D), ("bv", D), ("bp", D),
                          ("bf", F), ("bc", D)):
            bias_d[nm] = nc.dram_tensor(nm, [1, width], BF16,
                                        kind="ExternalInput")
    outT = nc.dram_tensor("outT", [D, T], F32, kind="ExternalOutput")
    outT_t = outT[:].rearrange("(o p) t -> p o t", p=128)

    # ---- constants (embedded in the NEFF) ----
    ones_bf = nc.inline_tensor(np.ones((1, T), ml_dtypes.bfloat16), "ones_bf")
    onescol = nc.inline_tensor(np.ones((128, 1), ml_dtypes.bfloat16),
                               "onescol")
    ident_np = np.eye(128, dtype=np.float32)
    ident_c = nc.inline_tensor(ident_np, "ident_c")
    # triangular mask for the diagonal 128x128 score blocks: 1 if i <= j
    m_np = (np.arange(128)[:, None] <= np.arange(128)[None, :]).astype(
        ml_dtypes.bfloat16)
    masks_d = nc.inline_tensor(m_np, "masks")

    with tile.TileContext(nc) as tc:
        with (
            tc.tile_pool(name="persist", bufs=1) as pp,
            tc.tile_pool(name="wts", bufs=3) as wpool,
            tc.tile_pool(name="work", bufs=3) as wkp,
            tc.tile_pool(name="small", bufs=1) as sp,
            tc.tile_pool(name="ps", bufs=8, space="PSUM") as ps,
        ):
            # ---- persistent SBUF tensors ----
            X = pp.tile([128, KC, T], F32, tag="X", name="X")
            ALN = pp.tile([128, KC, T], BF16, tag="ALN", name="ALN")
            QT = pp.tile([128, KC, T], BF16, tag="QT", name="QT")
            KT = pp.tile([128, KC, T], BF16, tag="KT", name="KT")
            Vt = pp.tile([128, NT, H * VS], BF16, tag="Vt", name="Vt")
            YT = pp.tile([128, KC, T], BF16, tag="YT", name="YT")
            X1 = pp.tile([128, KC, T], F32, tag="X1", name="X1")

            onesb_s = None
            if with_bias:
                onesb_s = pp.tile([1, T], BF16, tag="onesb", name="onesb_s")
                nc.sync.dma_start(out=onesb_s, in_=ones_bf[:])
            onescol_s = pp.tile([128, 1], BF16, tag="onescol",
                                name="onescol_s")
            nc.sync.dma_start(out=onescol_s, in_=onescol[:])
            ident_s = pp.tile([128, 128], F32, tag="ident", name="ident_s")
            nc.sync.dma_start(out=ident_s, in_=ident_c[:])
            eps_p = pp.tile([128, 1], F32, tag="eps", name="eps_p")
            nc.vector.memset(eps_p, eps)
            masks_s = pp.tile([128, 128], BF16, tag="masks", name="masks_s")
            nc.sync.dma_start(out=masks_s, in_=masks_d[:])
            biases = {}
            for nm, dten in bias_d.items():
                bt = pp.tile(list(dten.shape), BF16, tag=nm, name=f"{nm}_s")
                nc.sync.dma_start(out=bt, in_=dten[:])
                biases[nm] = bt

            Xbf = pp.tile([128, KC, T], BF16, tag="Xbf", name="Xbf")
            X1bf = pp.tile([128, KC, T], BF16, tag="X1bf", name="X1bf")

            # ---- load x^T ----
            xT_t = xT[:].rearrange("(o p) t -> p o t", p=128)
            for kc in range(KC):
                nc.sync.dma_start(out=X[:, kc, :], in_=xT_t[:, kc, :])
                nc.vector.tensor_copy(out=Xbf[:, kc, :], in_=X[:, kc, :])

            # ---- LayerNorm: dst = (src - mu) * rstd, cast bf16 ----
            # Per-token sums via ones-column matmuls; scalar math runs
            # 128-lane in token-major layout (PE transpose there and back);
            # GpSimd broadcasts the per-token factors across partitions.
            def layernorm_tci(src, srcbf, dst, tci):
                NJ = TS // 128
                if True:
                    tsl = slice(tci * TS, (tci + 1) * TS)
                    pmu = ps.tile([128, TS], F32, tag="ps", name="pmu")
                    psq = ps.tile([128, TS], F32, tag="ps", name="psq")
                    for kc in range(KC):
                        sqc = wkp.tile([128, TS], BF16, tag="sqc", bufs=3,
                                       name="sqc")
                        nc.scalar.activation(out=sqc, in_=srcbf[:, kc, tsl],
                                             func=AF.Square)
                        nc.tensor.matmul(
                            pmu[0:1, :], onescol_s[:], srcbf[:, kc, tsl],
                            start=(kc == 0), stop=(kc == KC - 1))
                        nc.tensor.matmul(
                            psq[0:1, :], onescol_s[:], sqc,
                            start=(kc == 0), stop=(kc == KC - 1))
                    # token-major stats for this half via PE transposes
                    stok = sp.tile([128, NJ, 2], F32, tag="stok", bufs=2,
                                   name="stok")
                    for s, pstat in ((0, pmu), (1, psq)):
                        srow = sp.tile([1, TS], F32, tag="srow", bufs=2,
                                       name="srow")
                        nc.vector.tensor_copy(out=srow, in_=pstat[0:1, :])
                        ptk = ps.tile([128, TS], F32, tag="ps", name="ptk")
                        for jj in range(NJ):
                            nc.tensor.transpose(
                                ptk[:, jj:jj + 1],
                                srow[0:1, jj * 128:(jj + 1) * 128],
                                ident_s[0:1, 0:1])
                        nc.vector.tensor_copy(out=stok[:, :, s],
                                              in_=ptk[:, 0:NJ])
                    nc.vector.tensor_scalar_mul(stok, stok, 1.0 / D)
                    mu = stok[:, :, 0]
                    m2 = stok[:, :, 1]
                    var_t = sp.tile([128, NJ], F32, tag="var_t", bufs=2,
                                    name="var_t")
                    nc.vector.tensor_tensor(var_t, mu, mu, OP.mult)
                    nc.vector.tensor_tensor(var_t, m2, var_t, OP.subtract)
                    nc.scalar.activation(out=var_t, in_=var_t, func=AF.Sqrt,
                                         bias=eps_p[:])
                    st2 = sp.tile([128, NJ, 2], F32, tag="st2", bufs=2,
                                  name="st2")
                    nc.vector.reciprocal_approx_fast(out=st2[:, :, 0],
                                                     in_=var_t)
                    nc.vector.tensor_tensor(st2[:, :, 1], mu, st2[:, :, 0],
                                            OP.mult)
                    nc.vector.tensor_scalar_mul(st2[:, :, 1], st2[:, :, 1],
                                                -1.0)
                    # back to row layout and broadcast across partitions
                    prow = ps.tile([128, TS], F32, tag="ps", name="prow")
                    nc.tensor.transpose(
                        prow[0:2 * NJ, 0:128],
                        st2.rearrange("p a b -> p (a b)"), ident_s[:])
                    rows16 = sp.tile([2 * NJ, 128], BF16, tag="rows16",
                                     bufs=2, name="rows16")
                    nc.vector.tensor_copy(out=rows16,
                                          in_=prow[0:2 * NJ, 0:128])
                    rows0 = sp.tile([1, 2 * NJ, 128], BF16, tag="rows",
                                    bufs=2, name="rows0")
                    nc.sync.dma_start(
                        out=rows0.rearrange("p a b -> p (a b)"),
                        in_=rows16[:])
                    # apply: dst = srcbf*rstd + (-mu*rstd), bf16 throughout
                    for jj in range(NJ):
                        j = tci * NJ + jj
                        tslj = slice(j * 128, (j + 1) * 128)
                        prep_r = wkp.tile([128, 128], BF16, tag="prep_r",
                                          bufs=2, name="prep_r")
                        nc.gpsimd.partition_broadcast(
                            prep_r, rows0[0:1, 2 * jj, :])
                        prep_n = wkp.tile([128, 128], BF16, tag="prep_n",
                                          bufs=2, name="prep_n")
                        nc.gpsimd.partition_broadcast(
                            prep_n, rows0[0:1, 2 * jj + 1, :])
                        tmp = wkp.tile([128, KC, 128], BF16, tag="lntmp",
                                       bufs=3, name="lntmp")
                        nc.vector.tensor_tensor(
                            tmp, srcbf[:, :, tslj],
                            prep_r[:, None, :].to_broadcast((128, KC, 128)),
                            OP.mult)
                        nc.vector.tensor_tensor(
                            dst[:, :, tslj], tmp,
                            prep_n[:, None, :].to_broadcast((128, KC, 128)),
                            OP.add)

            def layernorm(src, srcbf, dst):
                for tci in range(T // TS):
                    layernorm_tci(src, srcbf, dst, tci)

            layernorm(X, Xbf, ALN)

            # ---- QKV projections ----
            def bias_mm(psum, bias_t, msl, tsl):
                """Start `psum` with the rank-1 bias contribution; returns the
                start flag for the following contraction matmuls."""
                if bias_t is None:
                    return True
                nc.tensor.matmul(psum, bias_t[0:1, msl], onesb_s[0:1, tsl],
                                 start=True, stop=False)
                return False

            # V weights resident; Vt ones-columns
            wv_t = pp.tile([128, KC, D], BF16, tag="wv", name="wv_t")
            nc.sync.dma_start(out=wv_t, in_=wv_d[:])
            for h in range(H):
                nc.vector.memset(Vt[:, :, h * VS + HD: h * VS + HD + 1], 1.0)

            # ---- attention (software-pipelined with next pair's QKV) --
            # S^T[t, q] = sum_hd K^T[hd, t] Q^T[hd, q]  (K=HD contraction).
            # The PE stream is in-order, so PV matmuls that wait on the ACT
            # exp would stall everything behind them.  To keep the PE dense,
            # the NEXT head-pair's QKV/V matmuls (independent: they read only
            # ALN) are emitted as "filler units" interleaved between this
            # pair's score and PV matmuls.
            def softmax_norm(py, h, mc, half, qsl):
                """y^T[hd,q] = py[hd,q] / py[HD,q], written to YT — via
                GpSimd broadcast + DVE approx-reciprocal; no PE involvement."""
                hsl = slice(half * HD, (half + 1) * HD)
                den = wkp.tile([1, TQ], F32, tag="den", bufs=2, name="den")
                nc.vector.tensor_copy(out=den, in_=py[HD: HD + 1, :])
                rep = wkp.tile([HD, TQ], F32, tag="rep", bufs=3, name="rep")
                nc.gpsimd.partition_broadcast(rep, den[:])
                rrec = wkp.tile([HD, TQ], F32, tag="rrec", bufs=3, name="rrec")
                nc.vector.reciprocal_approx_fast(out=rrec, in_=rep)
                nc.vector.tensor_tensor(
                    YT[hsl, mc, qsl], py[:HD, :], rrec[:], OP.mult)

            def attn_scores(pys, mc, qc, tch):
                """score matmuls + exp + mask for one (pair, qc, key chunk);
                returns a closure emitting the matching PV matmuls."""
                tc_lo_diag = qc * TQ // 128
                tc_hi = (qc + 1) * TQ // 128
                t128 = slice(tch * 128, (tch + 1) * 128)
                dq = max(0, tch - tc_lo_diag) * 128
                rq = slice(dq, TQ)
                qslr = slice(qc * TQ + dq, (qc + 1) * TQ)
                pexps = []
                for half in range(HPC):
                    hsl = slice(half * HD, (half + 1) * HD)
                    psc = ps.tile([128, TQ], F32, tag="ps",
                                  name=f"psc{half}")
                    nc.tensor.matmul(
                        psc[:, rq], KT[hsl, mc, t128],
                        QT[hsl, mc, qslr], start=True, stop=True)
                    pexp = wkp.tile([128, TQ], BF16, tag="pexp",
                                    bufs=4, name="pexp")
                    nc.scalar.activation(out=pexp[:, rq], in_=psc[:, rq],
                                         func=AF.Exp)
                    if tch >= tc_lo_diag:
                        nc.vector.tensor_tensor(
                            pexp[:, dq:dq + 128],
                            pexp[:, dq:dq + 128], masks_s[:], OP.mult)
                    pexps.append(pexp)

                def emit_pv():
                    for half in range(HPC):
                        h = mc * HPC + half
                        nc.tensor.matmul(
                            pys[half][:VS, rq],
                            Vt[:, tch, h * VS: (h + 1) * VS],
                            pexps[half][:, rq],
                            start=(tch == 0), stop=(tch == tc_hi - 1))
                return emit_pv

            def make_filler(mc):
                """Filler units (closures) for pair mc's QKV + V matmuls."""
                msl = slice(mc * 128, (mc + 1) * 128)
                units = []
                for nm, wten, dstT in (("bq", wq_d, QT), ("bk", wk_d, KT)):
                    wt = wpool.tile([128, KC, 128], BF16, tag="w_qk", bufs=3,
                                    name="wt_qk")
                    nc.sync.dma_start(out=wt, in_=wten[mc])
                    for tci in range(T // TS):
                        def qkv_unit(nm=nm, wt=wt, dstT=dstT, tci=tci):
                            tsl = slice(tci * TS, (tci + 1) * TS)
                            pq = ps.tile([128, TS], F32, tag="ps", name="pq")
                            st = bias_mm(pq, biases.get(nm), msl, tsl)
                            for kc in range(KC):
                                nc.tensor.matmul(
                                    pq, wt[:, kc, :], ALN[:, kc, tsl],
                                    start=st and (kc == 0),
                                    stop=(kc == KC - 1))
                            nc.scalar.mul(dstT[:, mc, tsl], pq[:],
                                          scale if dstT is QT else 1.0)
                        units.append(qkv_unit)
                for tch in range(NT):
                    def v_unit(tch=tch):
                        t128 = slice(tch * 128, (tch + 1) * 128)
                        pv = ps.tile([128, TQ], F32, tag="ps", name="pv")
                        pvs = pv[:, 0:128]
                        st = True
                        if with_bias:
                            nc.tensor.matmul(pvs, onesb_s[0:1, 0:128],
                                             biases["bv"][0:1, msl],
                                             start=True, stop=False)
                            st = False
                        for kc in range(KC):
                            nc.tensor.matmul(
                                pvs, ALN[:, kc, t128], wv_t[:, kc, msl],
                                start=st and (kc == 0), stop=(kc == KC - 1))
                        dstv = Vt[:, tch, mc * HPC * VS: (mc + 1) * HPC * VS]
                        dstv = dstv.rearrange("p (h c) -> p h c",
                                              c=VS)[:, :, 0:HD]
                        nc.vector.tensor_copy(
                            out=dstv,
                            in_=pvs.rearrange("p (h c) -> p h c", c=HD))
                    units.append(v_unit)
                return units

            # V weights resident; Vt ones-columns
            wv_t = pp.tile([128, KC, D], BF16, tag="wv", name="wv_t")
            nc.sync.dma_start(out=wv_t, in_=wv_d[:])
            for h in range(H):
                nc.vector.memset(Vt[:, :, h * VS + HD: h * VS + HD + 1], 1.0)

            assert NQ in (1, 2)
            assert H // HPC == MC  # head-pair groups == feature chunks
            NTQ = TQ // 128
            NPAIR = H // HPC
            for u in make_filler(0):   # prologue: first pair's QKV/V
                u()
            for mc in range(NPAIR):
                filler = make_filler(mc + 1) if mc + 1 < NPAIR else []
                fi = 0
                pys = {qc: [ps.tile([128, TQ], F32, tag="ps",
                                    name=f"py{qc}_{half}")
                            for half in range(HPC)]
                       for qc in range(NQ)}
                for tch in range(NQ * NTQ):
                    pvs_cbs = []
                    if NQ == 2 and tch < NTQ:
                        pvs_cbs.append(attn_scores(pys[0], mc, 0, tch))
                    pvs_cbs.append(attn_scores(pys[NQ - 1], mc, NQ - 1, tch))
                    # independent PE work gives the exps time to finish
                    if fi < len(filler):
                        filler[fi](); fi += 1
                    for cb in pvs_cbs:
                        cb()
                    if NQ == 2 and tch == NTQ - 1:
                        for half in range(HPC):
                            softmax_norm(pys[0][half], mc * HPC + half, mc,
                                         half, slice(0, TQ))
                while fi < len(filler):
                    filler[fi](); fi += 1
                qc = NQ - 1
                qsl = slice(qc * TQ, (qc + 1) * TQ)
                for half in range(HPC):
                    softmax_norm(pys[qc][half], mc * HPC + half, mc,
                                 half, qsl)

            # ---- attn out-projection + residual (tci outer; each X1 half
            # feeds its LN2 half immediately, overlapping the next half) ----
            A2 = pp.tile([128, KC, T], BF16, tag="ALN", name="A2")
            for tci in range(T // TS):
                tsl = slice(tci * TS, (tci + 1) * TS)
                for mc in range(MC):
                    msl = slice(mc * 128, (mc + 1) * 128)
                    wt = wpool.tile([128, KC, 128], BF16, tag="w_p", bufs=3,
                                    name="wt_p")
                    nc.sync.dma_start(out=wt, in_=wp_d[mc])
                    po = ps.tile([128, TS], F32, tag="ps", name="po")
                    st = bias_mm(po, biases.get("bp"), msl, tsl)
                    for kc in range(KC):
                        nc.tensor.matmul(
                            po, wt[:, kc, :], YT[:, kc, tsl],
                            start=st and (kc == 0), stop=(kc == KC - 1))
                    nc.vector.tensor_tensor(
                        X1[:, mc, tsl], X[:, mc, tsl], po[:], OP.add)
                    nc.scalar.copy(out=X1bf[:, mc, tsl],
                                   in_=X1[:, mc, tsl])
                layernorm_tci(X1, X1bf, A2, tci)

            # ---- MLP: fc+gelu feeding cp accumulators, per 512-token half --
            # PSUM: MC pc accumulators held + 2 ph cycling = 8 banks exactly.
            for qc in range(T // TS):
                tsl = slice(qc * TS, (qc + 1) * TS)
                pcs = []
                for mc in range(MC):
                    pc = ps.tile([128, TS], F32, tag="ps", name=f"pc{mc}")
                    st = bias_mm(pc, biases.get("bc"),
                                 slice(mc * 128, (mc + 1) * 128), tsl)
                    pcs.append((pc, st))
                for fc in range(FC):
                    fsl = slice(fc * 128, (fc + 1) * 128)
                    wt = wpool.tile([128, KC, 128], BF16, tag="w_f", bufs=3,
                                    name="wt_f")
                    nc.sync.dma_start(out=wt, in_=wf_d[fc])
                    ph = ps.tile([128, TS], F32, tag="ps", name="ph")
                    st = bias_mm(ph, biases.get("bf"), fsl, tsl)
                    for kc in range(KC):
                        nc.tensor.matmul(
                            ph, wt[:, kc, :], A2[:, kc, tsl],
                            start=st and (kc == 0), stop=(kc == KC - 1))
                    hgel = wkp.tile([128, TS], BF16, tag="hgel", bufs=3,
                                    name="hgel")
                    nc.scalar.activation(out=hgel, in_=ph, func=gelu_func)
                    wtc = wpool.tile([128, MC, 128], BF16, tag="w_c", bufs=3,
                                     name="wt_c")
                    nc.sync.dma_start(out=wtc, in_=wc_d[fc])
                    for mc in range(MC):
                        pc, st = pcs[mc]
                        nc.tensor.matmul(
                            pc, wtc[:, mc, :], hgel,
                            start=st and (fc == 0), stop=(fc == FC - 1))
                for mc in range(MC):
                    pc, _ = pcs[mc]
                    ot = wkp.tile([128, TS], F32, tag="ot", bufs=3, name="ot")
                    nc.vector.tensor_tensor(ot, X1[:, mc, tsl], pc[:], OP.add)
                    nc.sync.dma_start(out=outT_t[:, mc, tsl], in_=ot)

    nc.finalize()
    return nc


# --------------------------------------------------------------------------
# Host-side input prep
# --------------------------------------------------------------------------
def _pack_lhsT(w):
    """[Dk, N] -> [N//128, 128, Dk//128, 128] contiguous lhsT tiles."""
    Dk, N = w.shape
    return np.ascontiguousarray(
        w.reshape(Dk // 128, 128, N // 128, 128).transpose(2, 1, 0, 3))


def prepare_weights(wq, bq, wk, bk, wv, bv, w_proj, b_proj, g1, be1, g2, be2,
                    w_fc, b_fc, w_cp, b_cp):
    """Fold LN affines + reshape heads; return packed bf16 arrays."""
    bf = ml_dtypes.bfloat16
    H_, D_, HD_ = wq.shape
    # [H, D, HD] -> [D, H*HD]
    wq2 = wq.transpose(1, 0, 2).reshape(D_, H_ * HD_).astype(np.float64)
    wk2 = wk.transpose(1, 0, 2).reshape(D_, H_ * HD_).astype(np.float64)
    wv2 = wv.transpose(1, 0, 2).reshape(D_, H_ * HD_).astype(np.float64)
    g1 = g1.astype(np.float64); be1 = be1.astype(np.float64)
    g2 = g2.astype(np.float64); be2 = be2.astype(np.float64)
    w_fc64 = w_fc.astype(np.float64)
    # fold LN affine: LN_aff(x) = n(x)*g + be  =>  W' = g[:,None]*W,
    # b' = b + be @ W
    arrs = {
        "wq": _pack_lhsT((g1[:, None] * wq2).astype(bf)),
        "wk": _pack_lhsT((g1[:, None] * wk2).astype(bf)),
        "wv": np.ascontiguousarray(
            (g1[:, None] * wv2).astype(bf)
            .reshape(-1, 128, wv2.shape[1]).transpose(1, 0, 2)),
        "wp": _pack_lhsT(w_proj.astype(bf)),
        "wf": _pack_lhsT((g2[:, None] * w_fc64).astype(bf)),
        "wc": np.ascontiguousarray(
            w_cp.astype(bf).reshape(-1, 128, w_cp.shape[1] // 128, 128)),
    }
    bias_arrs = {
        "bq": bq.reshape(-1).astype(np.float64) + be1 @ wq2,
        "bk": bk.reshape(-1).astype(np.float64) + be1 @ wk2,
        "bv": bv.reshape(-1).astype(np.float64) + be1 @ wv2,
        "bp": b_proj.astype(np.float64),
        "bf": b_fc.astype(np.float64) + be2 @ w_fc64,
        "bc": b_cp.astype(np.float64),
    }
    any_bias = bool(any(np.any(v != 0) for v in bias_arrs.values()))
    if any_bias:
        for k, v in bias_arrs.items():
            arrs[k] = v.astype(bf).reshape(1, -1)
    return arrs, any_bias


_NC_CACHE = {}


def kernel(**inputs):
    x = np.asarray(inputs["x"], np.float32)
    arrs, any_bias = prepare_weights(
        *(np.asarray(inputs[k]) for k in (
            "wq", "bq", "wk", "bk", "wv", "bv", "w_proj", "b_proj",
            "g1", "be1", "g2", "be2", "w_fc", "b_fc", "w_cp", "b_cp")))
    key = ("full", any_bias)
    if key not in _NC_CACHE:
        _NC_CACHE[key] = build_decoder_nc(with_bias=any_bias)
    nc = _NC_CACHE[key]

    in_maps = []
    for b in range(N_CORES):
        m = dict(arrs)
        m["xT"] = np.ascontiguousarray(x[b].T)
        in_maps.append(m)

    from concourse.bass_utils import run_bass_kernel_spmd
    res = run_bass_kernel_spmd(nc, in_maps, list(range(N_CORES)))
    out = np.stack([res.results[i]["outT"].T for i in range(N_CORES)])
    return out.astype(np.float32)


# revision 21
# speedup vs baseline: 1.2113x; 1.0170x over previous
"""Trainium2 Bass kernel for an nn.DecoderBlock (pre-LN GPT block).

Reference computation (per batch element, fp32):
    h  = LN(x; g1,be1);  q,k,v = per-head projections of h
    y  = causal-softmax(q k^T / sqrt(hd)) v ;  x1 = x + y @ w_proj + b_proj
    h2 = LN(x1; g2,be2); out = x1 + gelu_tanh(h2 @ w_fc + b_fc) @ w_cp + b_cp

Shapes: B=8, T=1024, D=768, H=12, HD=64, F=3072.

Strategy: pure data parallelism — batch element b runs on core b (B == n_cores
== 8); the decoder block is independent per batch element so no collectives are
needed.  On-chip, all activations are kept *feature-major* ([D, T]: features on
partitions, tokens on the free axis) so chained matmuls need no transposes:
    out^T[n, t] = sum_d W[d, n] * A^T[d, t]   (lhsT = W as stored, rhs = A^T)
Attention scores are computed transposed (S^T[t, q]) so the softmax-weighted
probabilities land directly in the [t, q] layout the P@V matmul needs as its
moving operand; the two heads sharing a 128-partition group issue their K=64
score matmuls back-to-back so the PE runs them concurrently in disjoint
row-groups.  The softmax denominator comes from augmenting V with a
ones-column (row HD of the PV output is sum_t P[t,q]).  Softmax max-subtraction
is skipped: post-LN scores are O(5) so fp32 exp cannot overflow.

The PE instruction stream in attention is pure matmuls: softmax normalization
runs entirely on GpSimd (partition_broadcast of the denominator row) + DVE
(reciprocal_approx_fast, multiply), so the PE never stalls on it.  LayerNorm
statistics are per-token sums gathered with ones-column matmuls; the per-token
scalar math runs 128-lane in a token-major layout reached via PE transposes,
and the results are broadcast across partitions by GpSimd.

Host-side prep (numpy): transpose x per core, fold LN affine (g,be) and all
biases into the weight matrices, pre-pack weights into DMA-contiguous tiles,
cast to bf16. Matmuls run in bf16 with fp32 PSUM accumulation; LN stats,
residuals and softmax denominators stay fp32.
"""

import numpy as np
import ml_dtypes

import concourse.bass as bass
import concourse.mybir as mybir
import concourse.tile as tile
from concourse import bacc

BF16 = mybir.dt.bfloat16
F32 = mybir.dt.float32
AF = mybir.ActivationFunctionType
OP = mybir.AluOpType

# Full-problem dimensions (hardcoded; harness contract).
B, T, D, H = 8, 1024, 768, 12
HD = D // H
F = 4 * D
EPS = 1e-5
N_CORES = 8


# --------------------------------------------------------------------------
# Bass program builder (parameterized so a small variant can be simulated)
# --------------------------------------------------------------------------
def build_decoder_nc(T=T, D=D, H=H, F=F, TQ=512, with_bias=False, eps=EPS,
                     gelu_func=AF.Gelu_apprx_tanh):
    """Build the single-core Bass program (same program runs SPMD on all cores).

    DRAM I/O layouts (all prepared host-side):
      xT    [D, T]             f32   x^T (feature-major)
      wq,wk [MC,128,KC,128]    bf16  packed lhsT tiles (LN1 affine folded in)
      wv    [128,KC,D]         bf16  rhs layout for token-major V
      wp    [MC,128,KC,128]    bf16  w_proj packed
      wf    [FC,128,KC,128]    bf16  w_fc packed (LN2 affine folded in)
      wc    [FC,128,MC,128]    bf16  w_cp packed fc-major (plain reshape)
      *_b   [1, N]             bf16  folded bias rows (only if with_bias)
      outT  [D, T]             f32   output^T
    """
    assert D % 128 == 0 and F % 128 == 0 and T % TQ == 0 and TQ % 128 == 0
    TS = min(512, T)           # token chunk for projections/LN stats
    assert T % TS == 0
    KC = D // 128          # contraction chunks over D
    FC = F // 128          # chunks over MLP hidden
    MC = D // 128          # output-feature chunks over D
    NT = T // 128          # key/token chunks of 128
    NQ = T // TQ           # query chunks of TQ
    ND = TQ // 128         # diagonal mask variants
    HPC = 128 // HD        # heads per 128-partition group (2 for HD=64)
    VS = HD + 1            # V columns per head incl. ones-column
    scale = 1.0 / np.sqrt(HD)
    assert H % HPC == 0 and 2 * NT <= 128

    nc = bacc.Bacc()

    # ---- DRAM I/O ----
    xT = nc.dram_tensor("xT", [D, T], F32, kind="ExternalInput")
    wq_d = nc.dram_tensor("wq", [MC, 128, KC, 128], BF16, kind="ExternalInput")
    wk_d = nc.dram_tensor("wk", [MC, 128, KC, 128], BF16, kind="ExternalInput")
    wv_d = nc.dram_tensor("wv", [128, KC, D], BF16, kind="ExternalInput")
    wp_d = nc.dram_tensor("wp", [MC, 128, KC, 128], BF16, kind="ExternalInput")
    wf_d = nc.dram_tensor("wf", [FC, 128, KC, 128], BF16, kind="ExternalInput")
    wc_d = nc.dram_tensor("wc", [FC, 128, MC, 128], BF16, kind="ExternalInput")
    bias_d = {}
    if with_bias:
        for nm, width in (("bq", D), ("bk", D), ("bv", D), ("bp", D),
                          ("bf", F), ("bc", D)):
            bias_d[nm] = nc.dram_tensor(nm, [1, width], BF16,
                                        kind="ExternalInput")
    outT = nc.dram_tensor("outT", [D, T], F32, kind="ExternalOutput")
    outT_t = outT[:].rearrange("(o p) t -> p o t", p=128)

    # ---- constants (embedded in the NEFF) ----
    ones_bf = nc.inline_tensor(np.ones((1, T), ml_dtypes.bfloat16), "ones_bf")
    onescol = nc.inline_tensor(np.ones((128, 1), ml_dtypes.bfloat16),
                               "onescol")
    ident_np = np.eye(128, dtype=np.float32)
    ident_c = nc.inline_tensor(ident_np, "ident_c")
    # triangular mask for the diagonal 128x128 score blocks: 1 if i <= j
    m_np = (np.arange(128)[:, None] <= np.arange(128)[None, :]).astype(
        ml_dtypes.bfloat16)
    masks_d = nc.inline_tensor(m_np, "masks")

    with tile.TileContext(nc) as tc:
        with (
            tc.tile_pool(name="persist", bufs=1) as pp,
            tc.tile_pool(name="wts", bufs=3) as wpool,
            tc.tile_pool(name="work", bufs=3) as wkp,
            tc.tile_pool(name="small", bufs=1) as sp,
            tc.tile_pool(name="ps", bufs=8, space="PSUM") as ps,
        ):
            # ---- persistent SBUF tensors ----
            X = pp.tile([128, KC, T], F32, tag="X", name="X")
            ALN = pp.tile([128, KC, T], BF16, tag="ALN", name="ALN")
            QT = pp.tile([128, KC, T], BF16, tag="QT", name="QT")
            KT = pp.tile([128, KC, T], BF16, tag="KT", name="KT")
            Vt = pp.tile([128, NT, H * VS], BF16, tag="Vt", name="Vt")
            YT = pp.tile([128, KC, T], BF16, tag="YT", name="YT")
            X1 = pp.tile([128, KC, T], F32, tag="X1", name="X1")

            onesb_s = None
            if with_bias:
                onesb_s = pp.tile([1, T], BF16, tag="onesb", name="onesb_s")
                nc.sync.dma_start(out=onesb_s, in_=ones_bf[:])
            onescol_s = pp.tile([128, 1], BF16, tag="onescol",
                                name="onescol_s")
            nc.sync.dma_start(out=onescol_s, in_=onescol[:])
            ident_s = pp.tile([128, 128], F32, tag="ident", name="ident_s")
            nc.sync.dma_start(out=ident_s, in_=ident_c[:])
            eps_p = pp.tile([128, 1], F32, tag="eps", name="eps_p")
            nc.vector.memset(eps_p, eps)
            masks_s = pp.tile([128, 128], BF16, tag="masks", name="masks_s")
            nc.sync.dma_start(out=masks_s, in_=masks_d[:])
            biases = {}
            for nm, dten in bias_d.items():
                bt = pp.tile(list(dten.shape), BF16, tag=nm, name=f"{nm}_s")
                nc.sync.dma_start(out=bt, in_=dten[:])
                biases[nm] = bt

            Xbf = pp.tile([128, KC, T], BF16, tag="Xbf", name="Xbf")
            X1bf = pp.tile([128, KC, T], BF16, tag="X1bf", name="X1bf")

            # ---- load x^T ----
            xT_t = xT[:].rearrange("(o p) t -> p o t", p=128)
            for kc in range(KC):
                nc.sync.dma_start(out=X[:, kc, :], in_=xT_t[:, kc, :])
                nc.vector.tensor_copy(out=Xbf[:, kc, :], in_=X[:, kc, :])

            # ---- LayerNorm: dst = (src - mu) * rstd, cast bf16 ----
            # Per-token sums via ones-column matmuls; scalar math runs
            # 128-lane in token-major layout (PE transpose there and back);
            # GpSimd broadcasts the per-token factors across partitions.
            def layernorm_tci(src, srcbf, dst, tci):
                NJ = TS // 128
                if True:
                    tsl = slice(tci * TS, (tci + 1) * TS)
                    pmu = ps.tile([128, TS], F32, tag="ps", name="pmu")
                    psq = ps.tile([128, TS], F32, tag="ps", name="psq")
                    for kc in range(KC):
                        sqc = wkp.tile([128, TS], BF16, tag="sqc", bufs=3,
                                       name="sqc")
                        nc.scalar.activation(out=sqc, in_=srcbf[:, kc, tsl],
                                             func=AF.Square)
                        nc.tensor.matmul(
                            pmu[0:1, :], onescol_s[:], srcbf[:, kc, tsl],
                            start=(kc == 0), stop=(kc == KC - 1))
                        nc.tensor.matmul(
                            psq[0:1, :], onescol_s[:], sqc,
                            start=(kc == 0), stop=(kc == KC - 1))
                    # token-major stats for this half via PE transposes
                    stok = sp.tile([128, NJ, 2], F32, tag="stok", bufs=2,
                                   name="stok")
                    for s, pstat in ((0, pmu), (1, psq)):
                        srow = sp.tile([1, TS], F32, tag="srow", bufs=2,
                                       name="srow")
                        nc.vector.tensor_copy(out=srow, in_=pstat[0:1, :])
                        ptk = ps.tile([128, TS], F32, tag="ps", name="ptk")
                        for jj in range(NJ):
                            nc.tensor.transpose(
                                ptk[:, jj:jj + 1],
                                srow[0:1, jj * 128:(jj + 1) * 128],
                                ident_s[0:1, 0:1])
                        nc.vector.tensor_copy(out=stok[:, :, s],
                                              in_=ptk[:, 0:NJ])
                    nc.vector.tensor_scalar_mul(stok, stok, 1.0 / D)
                    mu = stok[:, :, 0]
                    m2 = stok[:, :, 1]
                    var_t = sp.tile([128, NJ], F32, tag="var_t", bufs=2,
                                    name="var_t")
                    nc.vector.tensor_tensor(var_t, mu, mu, OP.mult)
                    nc.vector.tensor_tensor(var_t, m2, var_t, OP.subtract)
                    nc.scalar.activation(out=var_t, in_=var_t, func=AF.Sqrt,
                                         bias=eps_p[:])
                    st2 = sp.tile([128, NJ, 2], F32, tag="st2", bufs=2,
                                  name="st2")
                    nc.vector.reciprocal_approx_fast(out=st2[:, :, 0],
                                                     in_=var_t)
                    nc.vector.tensor_tensor(st2[:, :, 1], mu, st2[:, :, 0],
                                            OP.mult)
                    nc.vector.tensor_scalar_mul(st2[:, :, 1], st2[:, :, 1],
                                                -1.0)
                    # back to row layout and broadcast across partitions
                    prow = ps.tile([128, TS], F32, tag="ps", name="prow")
                    nc.tensor.transpose(
                        prow[0:2 * NJ, 0:128],
                        st2.rearrange("p a b -> p (a b)"), ident_s[:])
                    rows16 = sp.tile([2 * NJ, 128], BF16, tag="rows16",
                                     bufs=2, name="rows16")
                    nc.vector.tensor_copy(out=rows16,
                                          in_=prow[0:2 * NJ, 0:128])
                    rows0 = sp.tile([1, 2 * NJ, 128], BF16, tag="rows",
                                    bufs=2, name="rows0")
                    nc.sync.dma_start(
                        out=rows0.rearrange("p a b -> p (a b)"),
                        in_=rows16[:])
                    # apply: dst = srcbf*rstd + (-mu*rstd), bf16 throughout
                    for jj in range(NJ):
                        j = tci * NJ + jj
                        tslj = slice(j * 128, (j + 1) * 128)
                        prep_r = wkp.tile([128, 128], BF16, tag="prep_r",
                                          bufs=2, name="prep_r")
                        nc.gpsimd.partition_broadcast(
                            prep_r, rows0[0:1, 2 * jj, :])
                        prep_n = wkp.tile([128, 128], BF16, tag="prep_n",
                                          bufs=2, name="prep_n")
                        nc.gpsimd.partition_broadcast(
                            prep_n, rows0[0:1, 2 * jj + 1, :])
                        tmp = wkp.tile([128, KC, 128], BF16, tag="lntmp",
                                       bufs=3, name="lntmp")
                        nc.vector.tensor_tensor(
                            tmp, srcbf[:, :, tslj],
                            prep_r[:, None, :].to_broadcast((128, KC, 128)),
                            OP.mult)
                        nc.vector.tensor_tensor(
                            dst[:, :, tslj], tmp,
                            prep_n[:, None, :].to_broadcast((128, KC, 128)),
                            OP.add)

            def layernorm(src, srcbf, dst):
                for tci in range(T // TS):
                    layernorm_tci(src, srcbf, dst, tci)

            layernorm(X, Xbf, ALN)

            # ---- QKV projections ----
            def bias_mm(psum, bias_t, msl, tsl):
                """Start `psum` with the rank-1 bias contribution; returns the
                start flag for the following contraction matmuls."""
                if bias_t is None:
                    return True
                nc.tensor.matmul(psum, bias_t[0:1, msl], onesb_s[0:1, tsl],
                                 start=True, stop=False)
                return False

            # V weights resident; Vt ones-columns
            wv_t = pp.tile([128, KC, D], BF16, tag="wv", name="wv_t")
            nc.sync.dma_start(out=wv_t, in_=wv_d[:])
            for h in range(H):
                nc.vector.memset(Vt[:, :, h * VS + HD: h * VS + HD + 1], 1.0)

            # ---- attention (software-pipelined with next pair's QKV) --
            # S^T[t, q] = sum_hd K^T[hd, t] Q^T[hd, q]  (K=HD contraction).
            # The PE stream is in-order, so PV matmuls that wait on the ACT
            # exp would stall everything behind them.  To keep the PE dense,
            # the NEXT head-pair's QKV/V matmuls (independent: they read only
            # ALN) are emitted as "filler units" interleaved between this
            # pair's score and PV matmuls.
            def softmax_norm(py, h, mc, half, qsl):
                """y^T[hd,q] = py[hd,q] / py[HD,q], written to YT — via
                GpSimd broadcast + DVE approx-reciprocal; no PE involvement."""
                hsl = slice(half * HD, (half + 1) * HD)
                den = wkp.tile([1, TQ], F32, tag="den", bufs=2, name="den")
                nc.vector.tensor_copy(out=den, in_=py[HD: HD + 1, :])
                rep = wkp.tile([HD, TQ], F32, tag="rep", bufs=3, name="rep")
                nc.gpsimd.partition_broadcast(rep, den[:])
                rrec = wkp.tile([HD, TQ], F32, tag="rrec", bufs=3, name="rrec")
                nc.vector.reciprocal_approx_fast(out=rrec, in_=rep)
                nc.vector.tensor_tensor(
                    YT[hsl, mc, qsl], py[:HD, :], rrec[:], OP.mult)

            def attn_scores(pys, mc, qc, tch):
                """score matmuls + exp + mask for one (pair, qc, key chunk);
                returns a closure emitting the matching PV matmuls."""
                tc_lo_diag = qc * TQ // 128
                tc_hi = (qc + 1) * TQ // 128
                t128 = slice(tch * 128, (tch + 1) * 128)
                dq = max(0, tch - tc_lo_diag) * 128
                rq = slice(dq, TQ)
                qslr = slice(qc * TQ + dq, (qc + 1) * TQ)
                pexps = []
                for half in range(HPC):
                    hsl = slice(half * HD, (half + 1) * HD)
                    psc = ps.tile([128, TQ], F32, tag="ps",
                                  name=f"psc{half}")
                    nc.tensor.matmul(
                        psc[:, rq], KT[hsl, mc, t128],
                        QT[hsl, mc, qslr], start=True, stop=True)
                    pexp = wkp.tile([128, TQ], BF16, tag="pexp",
                                    bufs=4, name="pexp")
                    nc.scalar.activation(out=pexp[:, rq], in_=psc[:, rq],
                                         func=AF.Exp)
                    if tch >= tc_lo_diag:
                        nc.vector.tensor_tensor(
                            pexp[:, dq:dq + 128],
                            pexp[:, dq:dq + 128], masks_s[:], OP.mult)
                    pexps.append(pexp)

                def emit_pv():
                    for half in range(HPC):
                        h = mc * HPC + half
                        nc.tensor.matmul(
                            pys[half][:VS, rq],
                            Vt[:, tch, h * VS: (h + 1) * VS],
                            pexps[half][:, rq],
                            start=(tch == 0), stop=(tch == tc_hi - 1))
                return emit_pv

            def make_filler(mc):
                """Filler units (closures) for pair mc's QKV + V matmuls."""
                msl = slice(mc * 128, (mc + 1) * 128)
                units = []
                for nm, wten, dstT in (("bq", wq_d, QT), ("bk", wk_d, KT)):
                    wt = wpool.tile([128, KC, 128], BF16, tag="w_qk", bufs=3,
                                    name="wt_qk")
                    nc.sync.dma_start(out=wt, in_=wten[mc])
                    for tci in range(T // TS):
                        def qkv_unit(nm=nm, wt=wt, dstT=dstT, tci=tci):
                            tsl = slice(tci * TS, (tci + 1) * TS)
                            pq = ps.tile([128, TS], F32, tag="ps", name="pq")
                            st = bias_mm(pq, biases.get(nm), msl, tsl)
                            for kc in range(KC):
                                nc.tensor.matmul(
                                    pq, wt[:, kc, :], ALN[:, kc, tsl],
                                    start=st and (kc == 0),
                                    stop=(kc == KC - 1))
                            nc.scalar.mul(dstT[:, mc, tsl], pq[:],
                                          scale if dstT is QT else 1.0)
                        units.append(qkv_unit)
                for tch in range(NT):
                    def v_unit(tch=tch):
                        t128 = slice(tch * 128, (tch + 1) * 128)
                        pv = ps.tile([128, TQ], F32, tag="ps", name="pv")
                        pvs = pv[:, 0:128]
                        st = True
                        if with_bias:
                            nc.tensor.matmul(pvs, onesb_s[0:1, 0:128],
                                             biases["bv"][0:1, msl],
                                             start=True, stop=False)
                            st = False
                        for kc in range(KC):
                            nc.tensor.matmul(
                                pvs, ALN[:, kc, t128], wv_t[:, kc, msl],
                                start=st and (kc == 0), stop=(kc == KC - 1))
                        dstv = Vt[:, tch, mc * HPC * VS: (mc + 1) * HPC * VS]
                        dstv = dstv.rearrange("p (h c) -> p h c",
                                              c=VS)[:, :, 0:HD]
                        nc.vector.tensor_copy(
                            out=dstv,
                            in_=pvs.rearrange("p (h c) -> p h c", c=HD))
                    units.append(v_unit)
                return units

            # V weights resident; Vt ones-columns
            wv_t = pp.tile([128, KC, D], BF16, tag="wv", name="wv_t")
            nc.sync.dma_start(out=wv_t, in_=wv_d[:])
            for h in range(H):
                nc.vector.memset(Vt[:, :, h * VS + HD: h * VS + HD + 1], 1.0)

            A2 = pp.tile([128, KC, T], BF16, tag="ALN", name="A2")

            def make_proj_units(tci):
                """attn out-projection + residual for one token chunk, as
                filler-unit closures (one per output-feature chunk)."""
                tsl = slice(tci * TS, (tci + 1) * TS)
                units = []
                for mc in range(MC):
                    wt = wpool.tile([128, KC, 128], BF16, tag="w_p", bufs=3,
                                    name="wt_p")
                    nc.sync.dma_start(out=wt, in_=wp_d[mc])

                    def proj_unit(mc=mc, wt=wt):
                        msl = slice(mc * 128, (mc + 1) * 128)
                        po = ps.tile([128, TS], F32, tag="ps", name="po")
                        st = bias_mm(po, biases.get("bp"), msl, tsl)
                        for kc in range(KC):
                            nc.tensor.matmul(
                                po, wt[:, kc, :], YT[:, kc, tsl],
                                start=st and (kc == 0), stop=(kc == KC - 1))
                        nc.vector.tensor_tensor(
                            X1[:, mc, tsl], X[:, mc, tsl], po[:], OP.add)
                        nc.scalar.copy(out=X1bf[:, mc, tsl],
                                       in_=X1[:, mc, tsl])
                    units.append(proj_unit)
                return units

            assert NQ in (1, 2)
            assert H // HPC == MC  # head-pair groups == feature chunks
            NTQ = TQ // 128
            NPAIR = H // HPC
            for u in make_filler(0):   # prologue: first pair's QKV/V
                u()
            for mc in range(NPAIR):
                last = mc + 1 >= NPAIR
                # proj-as-filler needs the first token chunk == first query
                # chunk so that qc=0 norms are sufficient
                use_proj_filler = last and TS == TQ and NQ == 2
                filler = (make_proj_units(0) if use_proj_filler
                          else make_filler(mc + 1) if not last else [])
                fi = 0
                pys = {qc: [ps.tile([128, TQ], F32, tag="ps",
                                    name=f"py{qc}_{half}")
                            for half in range(HPC)]
                       for qc in range(NQ)}
                for tch in range(NQ * NTQ):
                    pvs_cbs = []
                    if NQ == 2 and tch < NTQ:
                        pvs_cbs.append(attn_scores(pys[0], mc, 0, tch))
                    pvs_cbs.append(attn_scores(pys[NQ - 1], mc, NQ - 1, tch))
                    # independent PE work gives the exps time to finish
                    # (for the last pair: proj of the first token chunk,
                    # legal only once its qc=0 normalizations are emitted)
                    if fi < len(filler) and not (use_proj_filler
                                                 and tch < NTQ):
                        filler[fi](); fi += 1
                    for cb in pvs_cbs:
                        cb()
                    if NQ == 2 and tch == NTQ - 1:
                        for half in range(HPC):
                            softmax_norm(pys[0][half], mc * HPC + half, mc,
                                         half, slice(0, TQ))
                while fi < len(filler):
                    filler[fi](); fi += 1
                qc = NQ - 1
                qsl = slice(qc * TQ, (qc + 1) * TQ)
                for half in range(HPC):
                    softmax_norm(pys[qc][half], mc * HPC + half, mc,
                                 half, qsl)

            # ---- remaining proj token chunks; each X1 half feeds its LN2
            # half immediately, overlapping the next chunk's matmuls ----
            first_tci = 1 if (TS == TQ and NQ == 2) else 0
            if first_tci == 1:
                layernorm_tci(X1, X1bf, A2, 0)
            for tci in range(first_tci, T // TS):
                for u in make_proj_units(tci):
                    u()
                layernorm_tci(X1, X1bf, A2, tci)

            # ---- MLP: fc+gelu feeding cp accumulators, per 512-token half --
            # PSUM: MC pc accumulators held + 2 ph cycling = 8 banks exactly.
            for qc in range(T // TS):
                tsl = slice(qc * TS, (qc + 1) * TS)
                pcs = []
                for mc in range(MC):
                    pc = ps.tile([128, TS], F32, tag="ps", name=f"pc{mc}")
                    st = bias_mm(pc, biases.get("bc"),
                                 slice(mc * 128, (mc + 1) * 128), tsl)
                    pcs.append((pc, st))
                for fc in range(FC):
                    fsl = slice(fc * 128, (fc + 1) * 128)
                    wt = wpool.tile([128, KC, 128], BF16, tag="w_f", bufs=3,
                                    name="wt_f")
                    nc.sync.dma_start(out=wt, in_=wf_d[fc])
                    ph = ps.tile([128, TS], F32, tag="ps", name="ph")
                    st = bias_mm(ph, biases.get("bf"), fsl, tsl)
                    for kc in range(KC):
                        nc.tensor.matmul(
                            ph, wt[:, kc, :], A2[:, kc, tsl],
                            start=st and (kc == 0), stop=(kc == KC - 1))
                    hgel = wkp.tile([128, TS], BF16, tag="hgel", bufs=3,
                                    name="hgel")
                    nc.scalar.activation(out=hgel, in_=ph, func=gelu_func)
                    wtc = wpool.tile([128, MC, 128], BF16, tag="w_c", bufs=3,
                                     name="wt_c")
                    nc.sync.dma_start(out=wtc, in_=wc_d[fc])
                    for mc in range(MC):
                        pc, st = pcs[mc]
                        nc.tensor.matmul(
                            pc, wtc[:, mc, :], hgel,
                            start=st and (fc == 0), stop=(fc == FC - 1))
                for mc in range(MC):
                    pc, _ = pcs[mc]
                    ot = wkp.tile([128, TS], F32, tag="ot", bufs=3, name="ot")
                    nc.vector.tensor_tensor(ot, X1[:, mc, tsl], pc[:], OP.add)
                    nc.sync.dma_start(out=outT_t[:, mc, tsl], in_=ot)

    nc.finalize()
    return nc


# --------------------------------------------------------------------------
# Host-side input prep
# --------------------------------------------------------------------------
def _pack_lhsT(w):
    """[Dk, N] -> [N//128, 128, Dk//128, 128] contiguous lhsT tiles."""
    Dk, N = w.shape
    return np.ascontiguousarray(
        w.reshape(Dk // 128, 128, N // 128, 128).transpose(2, 1, 0, 3))


def prepare_weights(wq, bq, wk, bk, wv, bv, w_proj, b_proj, g1, be1, g2, be2,
                    w_fc, b_fc, w_cp, b_cp):
    """Fold LN affines + reshape heads; return packed bf16 arrays."""
    bf = ml_dtypes.bfloat16
    H_, D_, HD_ = wq.shape
    # [H, D, HD] -> [D, H*HD]
    wq2 = wq.transpose(1, 0, 2).reshape(D_, H_ * HD_).astype(np.float64)
    wk2 = wk.transpose(1, 0, 2).reshape(D_, H_ * HD_).astype(np.float64)
    wv2 = wv.transpose(1, 0, 2).reshape(D_, H_ * HD_).astype(np.float64)
    g1 = g1.astype(np.float64); be1 = be1.astype(np.float64)
    g2 = g2.astype(np.float64); be2 = be2.astype(np.float64)
    w_fc64 = w_fc.astype(np.float64)
    # fold LN affine: LN_aff(x) = n(x)*g + be  =>  W' = g[:,None]*W,
    # b' = b + be @ W
    arrs = {
        "wq": _pack_lhsT((g1[:, None] * wq2).astype(bf)),
        "wk": _pack_lhsT((g1[:, None] * wk2).astype(bf)),
        "wv": np.ascontiguousarray(
            (g1[:, None] * wv2).astype(bf)
            .reshape(-1, 128, wv2.shape[1]).transpose(1, 0, 2)),
        "wp": _pack_lhsT(w_proj.astype(bf)),
        "wf": _pack_lhsT((g2[:, None] * w_fc64).astype(bf)),
        "wc": np.ascontiguousarray(
            w_cp.astype(bf).reshape(-1, 128, w_cp.shape[1] // 128, 128)),
    }
    bias_arrs = {
        "bq": bq.reshape(-1).astype(np.float64) + be1 @ wq2,
        "bk": bk.reshape(-1).astype(np.float64) + be1 @ wk2,
        "bv": bv.reshape(-1).astype(np.float64) + be1 @ wv2,
        "bp": b_proj.astype(np.float64),
        "bf": b_fc.astype(np.float64) + be2 @ w_fc64,
        "bc": b_cp.astype(np.float64),
    }
    any_bias = bool(any(np.any(v != 0) for v in bias_arrs.values()))
    if any_bias:
        for k, v in bias_arrs.items():
            arrs[k] = v.astype(bf).reshape(1, -1)
    return arrs, any_bias


_NC_CACHE = {}


def kernel(**inputs):
    x = np.asarray(inputs["x"], np.float32)
    arrs, any_bias = prepare_weights(
        *(np.asarray(inputs[k]) for k in (
            "wq", "bq", "wk", "bk", "wv", "bv", "w_proj", "b_proj",
            "g1", "be1", "g2", "be2", "w_fc", "b_fc", "w_cp", "b_cp")))
    key = ("full", any_bias)
    if key not in _NC_CACHE:
        _NC_CACHE[key] = build_decoder_nc(with_bias=any_bias)
    nc = _NC_CACHE[key]

    in_maps = []
    for b in range(N_CORES):
        m = dict(arrs)
        m["xT"] = np.ascontiguousarray(x[b].T)
        in_maps.append(m)

    from concourse.bass_utils import run_bass_kernel_spmd
    res = run_bass_kernel_spmd(nc, in_maps, list(range(N_CORES)))
    out = np.stack([res.results[i]["outT"].T for i in range(N_CORES)])
    return out.astype(np.float32)
